# revision 1
# baseline (speedup 1.0000x reference)
"""DiscreteMMSE Trainium2 kernel (v10).

Math (per batch b, sharded 4 batches/core over 8 cores):
  W = task_pool[:,:,0]                        # (T, D)
  pred = W @ x  (PE: fp16 hi/lo pair stacked K=128, exact to ~1e-5, 1 cyc/row)
  err  = pred - y  (PE: K=2 rank-2 matmul with [-y_hi; -y_lo], accumulated)
  sq   = 0.5*err^2            (ACT Square; some quads via DMA->SBUF + Pool mult)
  C    = exclusive cumsum_p sq (DVE tensor_tensor_scan, mask resets, fp32)
  m(p) = min_t C(t,p)          (Pool/DVE TT-min chain + PE transpose +
                                DVE free-reduce: exact per-point stabilizer)
  cs   = C - m                 (DVE TT add with materialized -m broadcast tile)
  e    = exp(-cs)              (ACT, f32r out)
  ws   = sum_t e(t,p)*[w_t|1]  (PE, f32r: TP stationary, (65,256) PSUM accum)
  out(p) = (x_p . ws[0:64,p]) / ws[64,p]
The per-column shift by m cancels exactly in the num/den ratio; cs >= 0 so
exp never overflows and den >= 1.  float32r (rounded ~11-bit mantissa) is used
only where relative 2^-11 error is harmless: e, TP, prod, the -m broadcast.

Sharding: data-parallel over batch: 32 batches -> 8 cores x 4. No collectives.
"""

import os
import sys

sys.path.insert(0, "/opt/trn_rl_repo")
sys.path.insert(0, "/opt/trn_rl_repo/concourse")

import numpy as np

import concourse.bass as bass
import concourse.tile as tile
from concourse import bacc, bass_utils, mybir, library_config

F32 = mybir.dt.float32
F32R = mybir.dt.float32r
F16 = mybir.dt.float16
AF = mybir.ActivationFunctionType
ALU = mybir.AluOpType
AX = mybir.AxisListType

B, P, D, T = 32, 256, 64, 4096
NCORES = 8
BLOC = B // NCORES          # 4 batches per core = 4 groups
NCH = T // 128              # 32 task chunks
NQ = NCH // 4               # 8 quads (4 chunks each) per group
SEG = 258                   # per-chunk scan segment: [pad, pad, sq0..sq255]
QW = 4 * SEG                # quad tile width (1032)
DA = D + 1

# tuning knobs
SQ_POOL = int(os.environ.get("KSQP", "0"))    # quads squared via DMA+Pool (of 32)
CHAIN_DVE = int(os.environ.get("KCHD", "7"))  # chain ops on DVE per group (of 7)
SUB_PE = int(os.environ.get("KSBP", "8"))     # subtract quads on PE (of 32)
DBG_G = int(os.environ.get("KDBG_G", "0"))
DBG_Q = int(os.environ.get("KDBG_Q", "0"))


def build_program(tc):
    nc = tc.nc

    wa_dram = nc.dram_tensor("w_aug", (DA, T), F32, kind="ExternalInput").ap()
    xn_dram = nc.dram_tensor("x_nat", (BLOC * P, D), F32, kind="ExternalInput").ap()
    tp_dram = nc.dram_tensor("tp_aug", (T, DA), F32, kind="ExternalInput").ap()
    xa_dram = nc.dram_tensor("x_aug", (DA, BLOC * P), F32, kind="ExternalInput").ap()
    id_dram = nc.dram_tensor("ident", (128, 128), F32, kind="ExternalInput").ap()
    on32_dram = nc.dram_tensor("ones_r32", (1, 128), F32, kind="ExternalInput").ap()
    out_dram = nc.dram_tensor("out", (BLOC, P), F32, kind="ExternalOutput").ap()

    from contextlib import ExitStack

    with ExitStack() as ctx:
        consts = ctx.enter_context(tc.tile_pool(name="consts", bufs=1))
        sqp = ctx.enter_context(tc.tile_pool(name="sqp", bufs=1))
        cp = ctx.enter_context(tc.tile_pool(name="cp", bufs=18))
        csp = ctx.enter_context(tc.tile_pool(name="csp", bufs=3))
        ep = ctx.enter_context(tc.tile_pool(name="ep", bufs=3))
        rmp = ctx.enter_context(tc.tile_pool(name="rmp", bufs=1))
        sm = ctx.enter_context(tc.tile_pool(name="sm", bufs=2))
        pq = ctx.enter_context(tc.tile_pool(name="pq", bufs=3, space="PSUM"))
        pp = ctx.enter_context(tc.tile_pool(name="pp", bufs=1, space="PSUM"))
        wsp = ctx.enter_context(tc.tile_pool(name="wsp", bufs=1, space="PSUM"))

        # ---- constants / inputs ----
        WA = consts.tile([DA, T], F32, tag="wa", name="wa")
        XA = consts.tile([DA, BLOC * P], F32, tag="xa", name="xa")
        XN = consts.tile([128, 2 * BLOC, D], F32, tag="xn", name="xn")
        ID = consts.tile([128, 128], F32, tag="ident", name="ident")
        TP_sb = consts.tile([128, NCH, DA], F32, tag="tpsb", name="tpsb")
        ONR32 = consts.tile([1, 128], F32, tag="onr32", name="onr32")

        nc.sync.dma_start(XA[:, 0:P], xa_dram[:, 0:P])
        for i8 in range(8):
            nc.sync.dma_start(WA[:, i8 * 512 : (i8 + 1) * 512],
                              wa_dram[:, i8 * 512 : (i8 + 1) * 512])
        nc.sync.dma_start(ID[:], id_dram)
        nc.sync.dma_start(ONR32[:], on32_dram)
        nc.sync.dma_start(XA[:, P:], xa_dram[:, P:])
        nc.sync.dma_start(XN[:], xn_dram.rearrange("(j q) d -> q j d", q=128))
        nc.sync.dma_start(TP_sb[:], tp_dram.rearrange("(c p) d -> p c d", p=128))

        mask = consts.tile([128, QW], F32, tag="mask", name="mask")
        nc.gpsimd.memset(mask[:], 1.0)
        for k in range(4):
            nc.gpsimd.memset(mask[:, k * SEG : k * SEG + 2], 0.0)

        # persistent sq quad ring (pad columns stay zero forever)
        sq_ring = []
        for i in range(3):
            t = sqp.tile([128, QW], F32, tag=f"sqr{i}", name=f"sqr{i}")
            for k in range(4):
                nc.gpsimd.memset(t[:, k * SEG : k * SEG + 2], 0.0)
            sq_ring.append(t)

        c_tiles = {}
        rm_state = {}
        mbc_t = {}
        ws_tiles = {}
        nsq = [0]
        nsub = [0]

        def p1_quad(g, q):
            # phase 1: err -> sq -> scan(C) -> running-min chain
            errq = pq.tile([128, 1024], F32, tag="eq", name="err")
            for k in range(4):
                c = 4 * q + k
                nc.tensor.matmul(
                    errq[:, k * 256 : (k + 1) * 256],
                    lhsT=WA[:, c * 128 : (c + 1) * 128],
                    rhs=XA[:, g * P : (g + 1) * P],
                    start=True,
                    stop=True,
                    skip_group_check=True,
                )
            sq = sq_ring[(g * NQ + q) % len(sq_ring)]
            sq_view = sq[:].rearrange("p (s x) -> p s x", x=SEG)[:, :, 2 : 2 + P]
            err_view = errq[:].rearrange("p (s x) -> p s x", x=P)
            nc.scalar.activation(
                sq_view, err_view, AF.Square, bias=0.0,
                scale=0.7071067811865476,
            )

            C = cp.tile([128, QW], F32, tag="c", name="c")
            c_tiles[(g, q)] = C
            nc.vector.tensor_tensor_scan(
                C[:], sq[:], mask[:], 0.0, op0=ALU.add, op1=ALU.mult
            )

            rmA, nA, firstC = rm_state[g]
            eng = nc.vector if (q - 1) < CHAIN_DVE else nc.gpsimd
            if nA == 0:
                rm_state[g] = (rmA, 1, C)
            elif nA == 1:
                eng.tensor_tensor(rmA[1][:], firstC[:], C[:], op=ALU.min)
                rm_state[g] = (rmA, 2, firstC)
            else:
                eng.tensor_tensor(
                    rmA[nA % 2][:], rmA[(nA + 1) % 2][:], C[:], op=ALU.min
                )
                rm_state[g] = (rmA, nA + 1, firstC)

        nfold_t = {}

        def p15a(g):
            # fold 4 chunk-sections of the runmin quad; negate
            rmA, nA, _ = rm_state[g]
            rmF = rmA[(nA + 1) % 2]
            rv = rmF[:].rearrange("p (s x) -> p s x", x=SEG)
            f01 = sm.tile([128, P], F32, tag="f01", name="f01")
            f23 = sm.tile([128, P], F32, tag="f23", name="f23")
            nc.vector.tensor_tensor(
                f01[:], rv[:, 0, 1 : 1 + P], rv[:, 1, 1 : 1 + P], op=ALU.min
            )
            nc.vector.tensor_tensor(
                f23[:], rv[:, 2, 1 : 1 + P], rv[:, 3, 1 : 1 + P], op=ALU.min
            )
            fold = sm.tile([128, P], F32, tag="fold", name="fold")
            nc.vector.tensor_tensor(fold[:], f01[:], f23[:], op=ALU.min)
            nfold = sm.tile([128, P], F32, tag=f"nfold{g % 2}", name="nfold")
            nc.scalar.activation(nfold[:], fold[:], AF.Copy, bias=0.0, scale=-1.0)
            nfold_t[g] = nfold

        def p15b(g):
            # partition-min via PE transposes + DVE free-axis max;
            # materialize the -m broadcast tile
            nfold = nfold_t.pop(g)
            nm2 = sm.tile([1, P], F32, tag="nm2", name="nm2", bufs=2)
            for h in range(2):
                tps = pp.tile([128, 128], F32, tag="pp", name="tps")
                nc.tensor.transpose(
                    tps[:, 0:128], nfold[:, h * 128 : (h + 1) * 128], ID[:]
                )
                mcol = sm.tile([128, 1], F32, tag="mcol", name="mcol")
                nc.vector.tensor_reduce(mcol[:], tps[:, 0:128], axis=AX.X, op=ALU.max)
                mrow = pp.tile([1, 128], F32, tag="pp", name="mrow")
                nc.tensor.transpose(mrow[:], mcol[:], ID[:])
                nc.scalar.copy(nm2[:, h * 128 : (h + 1) * 128], mrow[:])
            mb_ps = pp.tile([128, P], F32, tag="pp", name="mb_ps")
            nc.tensor.matmul(
                mb_ps[:], lhsT=ONR32[:], rhs=nm2[:], start=True, stop=True,
                skip_group_check=True,
            )
            mbc = sm.tile([128, P], F32, tag="mbc", name=f"mbc{g}")
            nc.scalar.copy(mbc[:], mb_ps[:])
            mbc_t[g] = (mbc, nm2)
            wsb = wsp.tile([128, 2 * DA], F32, tag="wsj", name=f"ws{g}")
            ws_tiles[g] = wsb

        def p2_quad(g, q):
            # phase 2: cs = C - m (DVE TT add, or PE ident+rank-1), e = exp(-cs)
            C = c_tiles.pop((g, q))
            Cv = C[:].rearrange("p (s x) -> p s x", x=SEG)[:, :, 1 : 1 + P]
            mbc, nm2 = mbc_t[g]
            j = nsub[0]
            nsub[0] += 1
            if (j * SUB_PE) // 32 != ((j + 1) * SUB_PE) // 32:
                cs = pq.tile([128, 1024], F32, tag="eq", name="cs_ps")
                for h in range(2):
                    nc.tensor.matmul(
                        cs[:, h * 512 : (h + 1) * 512],
                        lhsT=ID[:],
                        rhs=Cv[:, 2 * h : 2 * h + 2, :],
                        start=True,
                        stop=False,
                        skip_group_check=True,
                    )
                    for hh in range(2):
                        nc.tensor.matmul(
                            cs[:, h * 512 + hh * 256 : h * 512 + (hh + 1) * 256],
                            lhsT=ONR32[:],
                            rhs=nm2[:],
                            start=False,
                            stop=(hh == 1),
                            skip_group_check=True,
                        )
            else:
                cs = csp.tile([128, 1024], F32, tag="cs", name="cs")
                csv = cs[:].rearrange("p (s x) -> p s x", x=P)
                mbv = mbc[:].rearrange("p (a x) -> p a x", a=1).broadcast_to([128, 4, P])
                nc.vector.tensor_tensor(csv, Cv, mbv, op=ALU.add)
            e = ep.tile([128, 1024], F32, tag="e", name="e")
            nc.scalar.activation(e[:], cs[:], AF.Exp, bias=0.0, scale=-1.0)
            wsb = ws_tiles[g]
            for k in range(4):
                c = 4 * q + k
                for j in range(2):
                    nc.tensor.matmul(
                        wsb[:, j * DA : (j + 1) * DA],
                        lhsT=e[:, k * 256 + j * 128 : k * 256 + (j + 1) * 128],
                        rhs=TP_sb[:, c, :],
                        start=(c == 0 and j == 0),
                        stop=(c == NCH - 1),
                        skip_group_check=True,
                    )

        def p3(g):
            # out(q-lane, j) = (x . wsj[q, 0:64]) / wsj[q, 64]
            wsb = ws_tiles.pop(g)
            nrg = sm.tile([128, 2], F32, tag="nrg", name="nrg")
            dcol = sm.tile([128, 2], F32, tag="dcol", name="dcol")
            ws_sb = sm.tile([128, 2 * DA], F32, tag="ws_sb", name="ws_sb")
            nc.scalar.copy(ws_sb[:], wsb[:])
            for j in range(2):
                prod = sm.tile([128, D], F32, tag="prod", name="prod")
                nc.vector.tensor_tensor(
                    prod[:], XN[:, g * 2 + j, :], ws_sb[:, j * DA : j * DA + D],
                    op=ALU.mult,
                )
                nc.vector.tensor_reduce(
                    nrg[:, j : j + 1], prod[:], axis=AX.X, op=ALU.add
                )
                nc.vector.tensor_copy(
                    dcol[:, j : j + 1], ws_sb[:, j * DA + D : j * DA + D + 1]
                )
            rden = sm.tile([128, 2], F32, tag="rden", name="rden")
            nc.vector.reciprocal(rden[:], dcol[:])
            o = sm.tile([128, 2], F32, tag="o", name="o")
            nc.vector.tensor_tensor(o[:], nrg[:], rden[:], op=ALU.mult)
            nc.sync.dma_start(
                out_dram[g : g + 1, :].rearrange("b (h q) -> q (b h)", q=128),
                o[:],
            )

        def rm_alloc(g):
            rm_state[g] = (
                [rmp.tile([128, QW], F32, tag=f"rmA_{i}", name=f"rmA{g}_{i}",
                          bufs=2)
                 for i in (0, 1)],
                0,
                None,
            )

        # software-pipelined emission: interleave group g's phase-1 with
        # group g-1's phase-2; min-finalize (p15b) hides behind early quads
        EARLY = int(os.environ.get("KEARLY", "2"))
        for g in range(BLOC):
            rm_alloc(g)
            for q in range(NQ):
                p1_quad(g, q)
                if q == 0 and g > 0:
                    p15b(g - 1)
                if g > 0 and q >= EARLY:
                    p2_quad(g - 1, q - EARLY)
            if g > 0:
                for q in range(NQ - EARLY, NQ):
                    p2_quad(g - 1, q)
                p3(g - 1)
            p15a(g)
        p15b(BLOC - 1)
        for q in range(NQ):
            p2_quad(BLOC - 1, q)
        p3(BLOC - 1)


_CACHE = {}


def _get_nc():
    if "nc" not in _CACHE:
        nc = bacc.Bacc(
            "TRN2",
            target_bir_lowering=False,
            debug=False,
            enable_asserts=False,
            num_devices=NCORES,
        )
        with tile.TileContext(nc) as tc:
            build_program(tc)
        nc.compile()
        _CACHE["nc"] = nc
    return _CACHE["nc"]


def _split_pair(a):
    hi = a.astype(np.float16)
    lo = (a - hi.astype(np.float32)).astype(np.float16)
    return hi, lo


def _make_in_maps(data, targets, task_pool):
    data = np.ascontiguousarray(data, dtype=np.float32)
    targets = np.ascontiguousarray(targets, dtype=np.float32)
    task_pool = np.ascontiguousarray(task_pool, dtype=np.float32)
    W = task_pool[:, :, 0]  # (T, D)
    w_aug = np.concatenate([W.T, -np.ones((1, T), np.float32)], axis=0)
    tp_aug = np.concatenate([W, np.ones((T, 1), np.float32)], axis=1)  # (T, 65)
    ident = np.eye(128, dtype=np.float32)
    in_maps = []
    for core in range(NCORES):
        xa = np.empty((DA, BLOC * P), np.float32)
        for j in range(BLOC):
            b = core * BLOC + j
            xa[0:D, j * P : (j + 1) * P] = data[b].T
            xa[D, j * P : (j + 1) * P] = targets[b]
        xn = np.ascontiguousarray(
            data[core * BLOC : (core + 1) * BLOC].reshape(BLOC * P, D)
        )
        in_maps.append(
            {"w_aug": w_aug, "tp_aug": tp_aug, "x_aug": xa, "x_nat": xn,
             "ident": ident,
             "ones_r32": np.ones((1, 128), np.float32)}
        )
    return in_maps


def run(data, targets, task_pool, trace=False):
    nc = _get_nc()
    in_maps = _make_in_maps(data, targets, task_pool)
    res = bass_utils.run_bass_kernel_spmd(
        nc, in_maps, core_ids=list(range(NCORES)), trace=trace
    )
    out = np.empty((B, P), np.float32)
    for core in range(NCORES):
        out[core * BLOC : (core + 1) * BLOC] = res.results[core]["out"]
    return out, res


def kernel(data, targets, task_pool):
    out, _ = run(data, targets, task_pool)
    return out



# revision 26
# speedup vs baseline: 1.0675x; 1.0675x over previous
"""DiscreteMMSE Trainium2 kernel (v12).

Math (per batch b, sharded 4 batches/core over 8 cores):
  W = task_pool[:,:,0]                        # (T, D)
  err  = (W@x - y)/sqrt(2)   (PE fp16 hi/lo: [Wh;Wl]@[xh;xh] K=128 +
                              [Wh;1;1]@[xl;-yh;-yl] K=66; residual ~2^-22;
                              W,y pre-scaled by 1/sqrt(2) on host)
  sq   = err^2               (ACT Square, PSUM in -> SBUF out)
  nC   = cumsum_p (a - sq)   (DVE tensor_tensor_scan per chunk section:
                              state=(a+state)-sq; a(j)=(|x_j|^2+y_j^2)/2 is a
                              per-point rebase that cancels in the softmax but
                              keeps the fp32 scan state ~5x smaller = ~5x less
                              rounding noise than the reference's own cumsum)
  -m(p)= max_t nC(t,p)       (Pool: gpsimd.tensor_reduce(axis=C) per quad
                              into rows of a per-group stack tile, then
                              partition_all_reduce(max) across quads, then
                              3 tiny DVE folds over the 4 chunk sections and
                              gpsimd.partition_broadcast -> nmB (128,P))
  cs   = nC - (-m) = m - C   (DVE TT subtract, or PE fp32 ident + fp16
                              rank-1 (+m) into PSUM, by knob)
  e    = exp(+cs) fp16       (ACT, shifted: e[:,s,1:256]=exp(cs[:,s,0:255]),
                              col 0 preset to 1 == uniform posterior at p=0)
  ws   = sum_t e(t,p)*[w_t|1]  (PE fp16: TP (128,65) stationary, e moving,
                              (65,256) PSUM accum over 32 chunks)
  out(p) = (x_p . ws[0:64,p]) / ws[64,p]  (TT prod + ones-matmul + recip)
The shift by m cancels exactly in the num/den ratio; cs <= 0 so exp never
overflows and den >= 1.

Sharding: data-parallel over batch: 32 batches -> 8 cores x 4. No collectives.
"""

import os
import sys

sys.path.insert(0, "/opt/trn_rl_repo")
sys.path.insert(0, "/opt/trn_rl_repo/concourse")

import numpy as np

import concourse.bass as bass
import concourse.tile as tile
from concourse import bacc, bass_isa, bass_utils, mybir

F32 = mybir.dt.float32
F16 = mybir.dt.float16
AF = mybir.ActivationFunctionType
ALU = mybir.AluOpType
AX = mybir.AxisListType

B, P, D, T = 32, 256, 64, 4096
NCORES = 8
BLOC = B // NCORES          # 4 batches per core = 4 groups
NCH = T // 128              # 32 task chunks
NQ = NCH // 4               # 8 quads (4 chunks each) per group
QW = 4 * P                  # quad tile width (1024)
DA = D + 1

# tuning knobs
SUB_PE = int(os.environ.get("KSBP", "9"))    # quads (of 32) subtracted on PE
SQ_DMA = int(os.environ.get("KSQDMA", "0"))   # quads squared via DMA+DVE
EARLY = int(os.environ.get("KEARLY", "2"))
DBG = int(os.environ.get("KDBG", "0"))        # dump group-0 intermediates


def build_program(tc):
    nc = tc.nc

    whl_dram = nc.dram_tensor("whl", (128, T), F16, kind="ExternalInput").ap()
    wh2_dram = nc.dram_tensor("wh2", (66, T), F16, kind="ExternalInput").ap()
    xhh_dram = nc.dram_tensor("xhh", (128, BLOC * P), F16, kind="ExternalInput").ap()
    xl2_dram = nc.dram_tensor("xl2", (66, BLOC * P), F16, kind="ExternalInput").ap()
    xn_dram = nc.dram_tensor("x_nat", (BLOC * P, D), F32, kind="ExternalInput").ap()
    am_dram = nc.dram_tensor("amask", (128, BLOC * P), F32, kind="ExternalInput").ap()
    tp_dram = nc.dram_tensor("tp32", (T, DA), F32, kind="ExternalInput").ap()
    id_dram = nc.dram_tensor("ident", (128, 128), F32, kind="ExternalInput").ap()
    out_dram = nc.dram_tensor("out", (BLOC, P), F32, kind="ExternalOutput").ap()
    if DBG:
        dbg_nc = nc.dram_tensor("dbg_nc", (128, QW), F32, kind="ExternalOutput").ap()
        dbg_stk = nc.dram_tensor("dbg_stk", (NQ, QW), F32, kind="ExternalOutput").ap()
        dbg_nmb = nc.dram_tensor("dbg_nmb", (128, P), F32, kind="ExternalOutput").ap()
        dbg_e = nc.dram_tensor("dbg_e", (NQ, 128, QW), F32, kind="ExternalOutput").ap()
        dbg_ws = nc.dram_tensor("dbg_ws", (128, 2 * DA), F32, kind="ExternalOutput").ap()

    from contextlib import ExitStack

    with ExitStack() as ctx:
        consts = ctx.enter_context(tc.tile_pool(name="consts", bufs=1))
        sqp = ctx.enter_context(tc.tile_pool(name="sqp", bufs=3))
        cp = ctx.enter_context(tc.tile_pool(name="cp", bufs=12))
        stp = ctx.enter_context(tc.tile_pool(name="stp", bufs=2))
        csp = ctx.enter_context(tc.tile_pool(name="csp", bufs=3))
        ep = ctx.enter_context(tc.tile_pool(name="ep", bufs=3))
        sm = ctx.enter_context(tc.tile_pool(name="sm", bufs=2))
        pq = ctx.enter_context(tc.tile_pool(name="pq", bufs=2, space="PSUM"))
        wsp = ctx.enter_context(tc.tile_pool(name="wsp", bufs=2, space="PSUM"))

        # ---- constants / inputs ----
        WHL = consts.tile([128, T], F16, tag="whl", name="whl")
        WH2 = consts.tile([66, T], F16, tag="wh2", name="wh2")
        XHH = consts.tile([128, BLOC * P], F16, tag="xhh", name="xhh")
        XL2 = consts.tile([66, BLOC * P], F16, tag="xl2", name="xl2")
        TP_sb = consts.tile([128, NCH, DA], F32, tag="tpsb", name="tpsb")
        XN = consts.tile([128, 2 * BLOC, D], F32, tag="xn", name="xn")
        AM = consts.tile([128, BLOC * P], F32, tag="am", name="am")
        ID = consts.tile([128, 128], F32, tag="ident", name="ident")
        ONR = consts.tile([1, 128], F32, tag="onr", name="onr")


        nc.sync.dma_start(XHH[:], xhh_dram)
        nc.sync.dma_start(XL2[:], xl2_dram)
        for i8 in range(8):
            nc.sync.dma_start(WHL[:, i8 * 512 : (i8 + 1) * 512],
                              whl_dram[:, i8 * 512 : (i8 + 1) * 512])
        for i4 in range(4):
            nc.sync.dma_start(WH2[:, i4 * 1024 : (i4 + 1) * 1024],
                              wh2_dram[:, i4 * 1024 : (i4 + 1) * 1024])
        nc.sync.dma_start(TP_sb[:], tp_dram.rearrange("(c p) d -> p c d", p=128))
        nc.sync.dma_start(XN[:], xn_dram.rearrange("(j q) d -> q j d", q=128))
        nc.sync.dma_start(AM[:], am_dram)
        nc.sync.dma_start(ID[:], id_dram)
        nc.gpsimd.memset(ONR[:], 1.0)

        # e ring: fp16, col 0 of each 256-section preset to 1.0 (p=0 uniform)
        e_ring = []
        for i in range(3):
            t = ep.tile([128, QW], F32, tag=f"e{i}", name=f"e{i}")
            for k in range(4):
                nc.gpsimd.memset(t[:, k * P : k * P + 1], 1.0)
            e_ring.append(t)

        c_tiles = {}
        stack_t = {}
        nm_t = {}
        ws_tiles = {}
        cnt_sq = [0]
        cnt_sub = [0]

        def p1_quad(g, q):
            # phase 1: err -> sq -> scan(nC) -> Pool partition-max into stack
            errq = pq.tile([128, QW], F32, tag="eq", name="err")
            for k in range(4):
                c = 4 * q + k
                sl = slice(k * P, (k + 1) * P)
                nc.tensor.matmul(
                    errq[:, sl],
                    lhsT=WHL[:, c * 128 : (c + 1) * 128],
                    rhs=XHH[:, g * P : (g + 1) * P],
                    start=True, stop=False, skip_group_check=True,
                )
                nc.tensor.matmul(
                    errq[:, sl],
                    lhsT=WH2[:, c * 128 : (c + 1) * 128],
                    rhs=XL2[:, g * P : (g + 1) * P],
                    start=False, stop=True, skip_group_check=True,
                )
            sq = sqp.tile([128, QW], F32, tag="sq", name="sq")
            j = cnt_sq[0]
            cnt_sq[0] += 1
            if (j * SQ_DMA) // 32 != ((j + 1) * SQ_DMA) // 32:
                errS = sqp.tile([128, QW], F32, tag="errS", name="errS", bufs=2)
                nc.sync.dma_start(errS[:], errq[:])
                nc.vector.tensor_tensor(sq[:], errS[:], errS[:], op=ALU.mult)
            else:
                nc.scalar.activation(sq[:], errq[:], AF.Square, bias=0.0, scale=1.0)

            nC = cp.tile([128, QW], F32, tag="c", name="c")
            c_tiles[(g, q)] = nC
            amg = AM[:, g * P : (g + 1) * P]
            for s in range(4):
                nc.vector.tensor_tensor_scan(
                    nC[:, s * P : (s + 1) * P], amg, sq[:, s * P : (s + 1) * P],
                    0.0, op0=ALU.add, op1=ALU.subtract,
                )
            if DBG and g == 0 and q == 0:
                nc.sync.dma_start(dbg_nc, nC[:])
            # per-quad partition max (over the 128 tasks of each chunk row);
            # gpsimd C-reduce must write partition 0, so bounce via DMA into
            # the per-group stack row
            stk = stack_t[g]
            ctmp = stp.tile([1, QW], F32, tag="ctmp", name="ctmp", bufs=3)
            nc.gpsimd.tensor_reduce(ctmp[:], nC[:], axis=AX.C, op=ALU.max)
            nc.sync.dma_start(stk[q : q + 1, :], ctmp[:])

        def p15(g):
            # cross-quad max, fold 4 chunk-sections, broadcast -m
            stk = stack_t[g]
            stk2 = stp.tile([NQ, QW], F32, tag="stk2", name=f"stk2_{g}")
            nc.gpsimd.partition_all_reduce(
                stk2[:], stk[:], channels=NQ, reduce_op=bass_isa.ReduceOp.max
            )
            sv = stk2[0:1, :].rearrange("p (s x) -> p s x", x=P)
            f01 = sm.tile([1, P], F32, tag="f01", name="f01")
            f23 = sm.tile([1, P], F32, tag="f23", name="f23")
            nc.vector.tensor_tensor(f01[:], sv[:, 0, :], sv[:, 1, :], op=ALU.max)
            nc.vector.tensor_tensor(f23[:], sv[:, 2, :], sv[:, 3, :], op=ALU.max)
            nm1 = sm.tile([1, P], F32, tag="nm1", name="nm1")
            nc.vector.tensor_tensor(nm1[:], f01[:], f23[:], op=ALU.max)
            nmB = sm.tile([128, P], F32, tag=f"nmB{g % 2}", name=f"nmB{g}")
            nc.gpsimd.partition_broadcast(nmB[:], nm1[:], channels=128)
            mh = None
            if SUB_PE > 0:
                mh = sm.tile([1, P], F32, tag=f"mh{g % 2}", name=f"mh{g}")
                nc.vector.tensor_scalar_mul(mh[:], nm1[:], -1.0)
            nm_t[g] = (nmB, mh)
            if DBG and g == 0:
                nc.sync.dma_start(dbg_stk, stk2[:])
                nc.sync.dma_start(dbg_nmb, nmB[:])
            wsb = wsp.tile([128, 2 * DA], F32, tag="wsj", name=f"ws{g}")
            ws_tiles[g] = wsb

        def p2_quad(g, q):
            # phase 2: cs = nC - (-m) = m - C; e = exp(cs) shifted; ws accum
            nC = c_tiles.pop((g, q))
            nmB, mh = nm_t[g]
            j = cnt_sub[0]
            cnt_sub[0] += 1
            on_pe = (j * SUB_PE) // 32 != ((j + 1) * SUB_PE) // 32
            if on_pe:
                cs = pq.tile([128, QW], F32, tag="eq", name="cs_ps")
                Cv = nC[:].rearrange("p (s x) -> p s x", x=P)
                for h in range(2):
                    sl = slice(h * 512, (h + 1) * 512)
                    nc.tensor.matmul(
                        cs[:, sl], lhsT=ID[:], rhs=Cv[:, 2 * h : 2 * h + 2, :],
                        start=True, stop=False, skip_group_check=True,
                    )
                    for hh in range(2):
                        nc.tensor.matmul(
                            cs[:, (2 * h + hh) * P : (2 * h + hh + 1) * P],
                            lhsT=ONR[:], rhs=mh[:],
                            start=False, stop=(hh == 1), skip_group_check=True,
                        )
            else:
                cs = csp.tile([128, QW], F32, tag="cs", name="cs")
                csv = cs[:].rearrange("p (s x) -> p s x", x=P)
                Cv = nC[:].rearrange("p (s x) -> p s x", x=P)
                nmv = (nmB[:].rearrange("p (a x) -> p a x", a=1)
                       .broadcast_to([128, 4, P]))
                nc.vector.tensor_tensor(csv, Cv, nmv, op=ALU.subtract)
            e = e_ring[(g * NQ + q) % len(e_ring)]
            ev = e[:].rearrange("p (s x) -> p s x", x=P)[:, :, 1:P]
            csv2 = cs[:].rearrange("p (s x) -> p s x", x=P)[:, :, 0 : P - 1]
            nc.scalar.activation(ev, csv2, AF.Exp, bias=0.0, scale=1.0)
            if DBG and g == 0:
                nc.sync.dma_start(dbg_e[q], e[:])
            wsb = ws_tiles[g]
            for k in range(4):
                c = 4 * q + k
                for j in range(2):
                    nc.tensor.matmul(
                        wsb[:, j * DA : (j + 1) * DA],
                        lhsT=e[:, k * P + j * 128 : k * P + (j + 1) * 128],
                        rhs=TP_sb[:, c, :],
                        start=(c == 0 and j == 0), stop=(c == NCH - 1),
                        skip_group_check=True,
                    )

        def p3(g):
            # out(q-lane, j) = (x . wsj[q, 0:64]) / wsj[q, 64]
            wsb = ws_tiles.pop(g)
            if DBG and g == 0:
                ws_dbg = sm.tile([128, 2 * DA], F32, tag="wsdbg", name="wsdbg")
                nc.vector.tensor_copy(ws_dbg[:], wsb[:])
                nc.sync.dma_start(dbg_ws, ws_dbg[:])
            nrg = sm.tile([128, 2], F32, tag="nrg", name="nrg")
            dcol = sm.tile([128, 2], F32, tag="dcol", name="dcol")
            for j in range(2):
                prod = sm.tile([128, D], F32, tag="prod", name="prod")
                nc.vector.tensor_tensor(
                    prod[:], XN[:, g * 2 + j, :], wsb[:, j * DA : j * DA + D],
                    op=ALU.mult,
                )
                nc.vector.tensor_reduce(
                    nrg[:, j : j + 1], prod[:], axis=AX.X, op=ALU.add
                )
                nc.vector.tensor_copy(
                    dcol[:, j : j + 1], wsb[:, j * DA + D : j * DA + D + 1]
                )
            rden = sm.tile([128, 2], F32, tag="rden", name="rden")
            nc.vector.reciprocal(rden[:], dcol[:])
            o = sm.tile([128, 2], F32, tag="o", name="o")
            nc.vector.tensor_tensor(o[:], nrg[:], rden[:], op=ALU.mult)
            nc.sync.dma_start(
                out_dram[g : g + 1, :].rearrange("b (h q) -> q (b h)", q=128),
                o[:],
            )

        # software-pipelined emission: interleave group g's phase-1 with
        # group g-1's phase-2; min-finalize (p15) hides behind early quads
        for g in range(BLOC):
            stack_t[g] = stp.tile([NQ, QW], F32, tag="stk", name=f"stk{g}")
            for q in range(NQ):
                p1_quad(g, q)
                if q == 0 and g > 0:
                    p15(g - 1)
                if g > 0 and q >= EARLY:
                    p2_quad(g - 1, q - EARLY)
            if g > 0:
                for q in range(NQ - EARLY, NQ):
                    p2_quad(g - 1, q)
                p3(g - 1)
        p15(BLOC - 1)
        for q in range(NQ):
            p2_quad(BLOC - 1, q)
        p3(BLOC - 1)


_CACHE = {}


def _get_nc():
    if "nc" not in _CACHE:
        nc = bacc.Bacc(
            "TRN2",
            target_bir_lowering=False,
            debug=False,
            enable_asserts=False,
            num_devices=NCORES,
        )
        with tile.TileContext(nc) as tc:
            build_program(tc)
        nc.compile()
        _CACHE["nc"] = nc
    return _CACHE["nc"]


def _split_pair(a):
    hi = a.astype(np.float16)
    lo = (a - hi.astype(np.float32)).astype(np.float16)
    return hi, lo


def _make_in_maps(data, targets, task_pool):
    data = np.ascontiguousarray(data, dtype=np.float32)
    targets = np.ascontiguousarray(targets, dtype=np.float32)
    task_pool = np.ascontiguousarray(task_pool, dtype=np.float32)
    isq2 = np.float32(1.0 / np.sqrt(2.0))
    W = task_pool[:, :, 0]  # (T, D)
    Ws = W.T * isq2  # (D, T), pre-scaled so sq = err^2 directly
    wh, wl = _split_pair(Ws)
    whl = np.concatenate([wh, wl], axis=0)  # (128, T) fp16
    wh2 = np.concatenate(
        [wh, np.ones((2, T), np.float16)], axis=0
    )  # (66, T): [Wh; 1; 1]
    tp32 = np.concatenate(
        [W, np.ones((T, 1), np.float32)], axis=1
    )  # (T, 65) fp32
    ident = np.eye(128, dtype=np.float32)
    Wsub = W[::16]  # (256, D) deterministic subsample for lambda estimate
    in_maps = []
    for core in range(NCORES):
        xs = np.empty((D, BLOC * P), np.float32)
        ys = np.empty((BLOC * P,), np.float32)
        for j in range(BLOC):
            b = core * BLOC + j
            xs[:, j * P : (j + 1) * P] = data[b].T
            ys[j * P : (j + 1) * P] = targets[b]
        xh, xl = _split_pair(xs)
        xhh = np.concatenate([xh, xh], axis=0)  # (128, 1024)
        nys = -ys * isq2
        nyh, nyl = _split_pair(nys)
        xl2 = np.concatenate(
            [xl, nyh[None, :], nyl[None, :]], axis=0
        )  # (66, 1024): [xl; -yh; -yl]
        xn = np.ascontiguousarray(
            data[core * BLOC : (core + 1) * BLOC].reshape(BLOC * P, D)
        )
        av = 0.5 * ((xn ** 2).sum(axis=1) + ys ** 2).astype(np.float32)
        # winner-targeted rebase: scale a by lambda ~= C_min/A (per batch,
        # estimated from a task subsample) so the scan state stays small for
        # the low-C tasks that dominate the posterior -> ~5x less fp32
        # rounding noise where it matters
        for j in range(BLOC):
            b = core * BLOC + j
            es = Wsub @ data[b].T - targets[b][None, :]
            Cs = 0.5 * (es ** 2).sum(axis=1)
            Ab = av[j * P : (j + 1) * P].sum()
            lam = np.float32(Cs.min() / (2.0 * Ab))
            av[j * P : (j + 1) * P] *= 2.0 * lam
        amask = np.broadcast_to(av[None, :], (128, BLOC * P)).copy()
        in_maps.append(
            {"whl": whl, "wh2": wh2, "xhh": xhh, "xl2": xl2, "tp32": tp32,
             "x_nat": xn, "amask": amask, "ident": ident}
        )
    return in_maps


def run(data, targets, task_pool, trace=False):
    nc = _get_nc()
    in_maps = _make_in_maps(data, targets, task_pool)
    res = bass_utils.run_bass_kernel_spmd(
        nc, in_maps, core_ids=list(range(NCORES)), trace=trace
    )
    out = np.empty((B, P), np.float32)
    for core in range(NCORES):
        out[core * BLOC : (core + 1) * BLOC] = res.results[core]["out"]
    return out, res


def kernel(data, targets, task_pool):
    out, _ = run(data, targets, task_pool)
    return out


# revision 39
# speedup vs baseline: 1.2955x; 1.2135x over previous
"""DiscreteMMSE Trainium2 kernel (v12).

Math (per batch b, sharded 4 batches/core over 8 cores):
  W = task_pool[:,:,0]                        # (T, D)
  err  = (W@x - y)/sqrt(2)   (PE fp16 hi/lo: [Wh;Wl]@[xh;xh] K=128 +
                              [Wh;1;1]@[xl;-yh;-yl] K=66; residual ~2^-22;
                              W,y pre-scaled by 1/sqrt(2) on host)
  sq   = err^2               (ACT Square, PSUM in -> SBUF out)
  nC   = cumsum_p (a - sq)   (DVE tensor_tensor_scan per chunk section:
                              state=(a+state)-sq; a(j)=(|x_j|^2+y_j^2)/2 is a
                              per-point rebase that cancels in the softmax but
                              keeps the fp32 scan state ~5x smaller = ~5x less
                              rounding noise than the reference's own cumsum)
  -m(p)= max_t nC(t,p)       (Pool: gpsimd.tensor_reduce(axis=C) per quad
                              into rows of a per-group stack tile, then
                              partition_all_reduce(max) across quads, then
                              3 tiny DVE folds over the 4 chunk sections and
                              gpsimd.partition_broadcast -> nmB (128,P))
  cs   = nC - (-m) = m - C   (DVE TT subtract, or PE fp32 ident + fp16
                              rank-1 (+m) into PSUM, by knob)
  e    = exp(+cs) fp16       (ACT, shifted: e[:,s,1:256]=exp(cs[:,s,0:255]),
                              col 0 preset to 1 == uniform posterior at p=0)
  ws   = sum_t e(t,p)*[w_t|1]  (PE fp16: TP (128,65) stationary, e moving,
                              (65,256) PSUM accum over 32 chunks)
  out(p) = (x_p . ws[0:64,p]) / ws[64,p]  (TT prod + ones-matmul + recip)
The shift by m cancels exactly in the num/den ratio; cs <= 0 so exp never
overflows and den >= 1.

Sharding: data-parallel over batch: 32 batches -> 8 cores x 4. No collectives.
"""

import os
import sys

sys.path.insert(0, "/opt/trn_rl_repo")
sys.path.insert(0, "/opt/trn_rl_repo/concourse")

import numpy as np

import concourse.bass as bass
import concourse.tile as tile
from concourse import bacc, bass_isa, bass_utils, mybir

F32 = mybir.dt.float32
F16 = mybir.dt.float16
AF = mybir.ActivationFunctionType
ALU = mybir.AluOpType
AX = mybir.AxisListType

B, P, D, T = 32, 256, 64, 4096
NCORES = 8
BLOC = B // NCORES          # 4 batches per core = 4 groups
NCH = T // 128              # 32 task chunks
NQ = NCH // 4               # 8 quads (4 chunks each) per group
QW = 4 * P                  # quad tile width (1024)
DA = D + 1

# tuning knobs
SUB_PE = int(os.environ.get("KSBP", "7"))    # quads (of 32) subtracted on PE
SQ_DMA = int(os.environ.get("KSQDMA", "0"))   # quads squared via DMA+DVE
EARLY = int(os.environ.get("KEARLY", "3"))
PRIO_MIN = int(os.environ.get("KPRIO", "40"))
DBG = int(os.environ.get("KDBG", "0"))        # dump group-0 intermediates


def build_program(tc):
    nc = tc.nc

    whl_dram = nc.dram_tensor("whl", (128, T), F16, kind="ExternalInput").ap()
    wh2_dram = nc.dram_tensor("wh2", (66, T), F16, kind="ExternalInput").ap()
    xhh_dram = nc.dram_tensor("xhh", (128, BLOC * P), F16, kind="ExternalInput").ap()
    xl2_dram = nc.dram_tensor("xl2", (66, BLOC * P), F16, kind="ExternalInput").ap()
    xn_dram = nc.dram_tensor("x_nat", (128, 2 * BLOC * D), F32, kind="ExternalInput").ap()
    am_dram = nc.dram_tensor("amask", (128, BLOC * P), F32, kind="ExternalInput").ap()
    tp_dram = nc.dram_tensor("tp32", (128, NCH * DA), F32, kind="ExternalInput").ap()
    id_dram = nc.dram_tensor("ident", (128, 128), F32, kind="ExternalInput").ap()
    out_dram = nc.dram_tensor("out", (BLOC, P), F32, kind="ExternalOutput").ap()
    if DBG:
        dbg_nc = nc.dram_tensor("dbg_nc", (128, QW), F32, kind="ExternalOutput").ap()
        dbg_stk = nc.dram_tensor("dbg_stk", (NQ, QW), F32, kind="ExternalOutput").ap()
        dbg_nmb = nc.dram_tensor("dbg_nmb", (128, P), F32, kind="ExternalOutput").ap()
        dbg_e = nc.dram_tensor("dbg_e", (NQ, 128, QW), F32, kind="ExternalOutput").ap()
        dbg_ws = nc.dram_tensor("dbg_ws", (128, 2 * DA), F32, kind="ExternalOutput").ap()

    from contextlib import ExitStack

    with ExitStack() as ctx:
        consts = ctx.enter_context(tc.tile_pool(name="consts", bufs=1))
        sqp = ctx.enter_context(tc.tile_pool(name="sqp", bufs=3))
        cp = ctx.enter_context(tc.tile_pool(name="cp", bufs=12))
        stp = ctx.enter_context(tc.tile_pool(name="stp", bufs=2))
        csp = ctx.enter_context(tc.tile_pool(name="csp", bufs=3))
        ep = ctx.enter_context(tc.tile_pool(name="ep", bufs=3))
        sm = ctx.enter_context(tc.tile_pool(name="sm", bufs=2))
        pq = ctx.enter_context(tc.tile_pool(name="pq", bufs=3, space="PSUM"))
        wsp = ctx.enter_context(tc.tile_pool(name="wsp", bufs=2, space="PSUM"))

        # ---- constants / inputs ----
        WHL = consts.tile([128, T], F16, tag="whl", name="whl")
        WH2 = consts.tile([66, T], F16, tag="wh2", name="wh2")
        XHH = consts.tile([128, BLOC * P], F16, tag="xhh", name="xhh")
        XL2 = consts.tile([66, BLOC * P], F16, tag="xl2", name="xl2")
        TP_sb = consts.tile([128, NCH, DA], F32, tag="tpsb", name="tpsb")
        XN = consts.tile([128, 2 * BLOC, D], F32, tag="xn", name="xn")
        AMg = [consts.tile([128, P], F32, tag=f"am{g}", name=f"am{g}")
               for g in range(BLOC)]
        ID = consts.tile([128, 128], F32, tag="ident", name="ident")
        ONR = consts.tile([1, 128], F32, tag="onr", name="onr")


        # ordered by first use: quad (g0,q0) needs XHH/XL2/WHL0/WH20, the
        # first scan needs AM; bulk/late tensors go via the ACT DGE queue so
        # the SP sequencer is free for the per-quad stack-bounce DMAs
        # HWDGE and the DMA engines serialize in acquire order, so order the
        # DMAs by first use: mm1(g0,q0) needs XHH+WHL cols 0:1024, mm2 needs
        # XL2+WH2, the first scan needs AM[g0]
        nc.sync.dma_start(XHH[:], xhh_dram)
        nc.sync.dma_start(WHL[:, 0:1024], whl_dram[:, 0:1024])
        nc.sync.dma_start(XL2[:], xl2_dram)
        nc.sync.dma_start(WH2[:, 0:1024], wh2_dram[:, 0:1024])
        nc.sync.dma_start(AMg[0][:], am_dram[:, 0:P])
        nc.sync.dma_start(WHL[:, 1024:], whl_dram[:, 1024:])
        nc.sync.dma_start(WH2[:, 1024:], wh2_dram[:, 1024:])
        for g in range(1, BLOC):
            nc.sync.dma_start(AMg[g][:], am_dram[:, g * P : (g + 1) * P])
        nc.gpsimd.memset(ONR[:], 1.0)
        late_dma = [False]

        def emit_late_dmas():
            # ID/XN/TP are first needed at p15(0)/p2(0)/p3(0); emitting them
            # here (mid group 0) keeps the ACT sequencer free for the first
            # squares at startup
            if late_dma[0]:
                return
            late_dma[0] = True
            with tc.tile_wait_until(0.012):
                nc.scalar.dma_start(ID[:], id_dram)
                nc.scalar.dma_start(
                    XN[:], xn_dram.rearrange("q (j d) -> q j d", d=D))
                nc.scalar.dma_start(
                    TP_sb[:], tp_dram.rearrange("q (c d) -> q c d", d=DA))

        # e ring: fp16, col 0 of each 256-section preset to 1.0 (p=0 uniform)
        e_ring = []
        for i in range(3):
            t = ep.tile([128, QW], F32, tag=f"e{i}", name=f"e{i}")
            for k in range(4):
                nc.gpsimd.memset(t[:, k * P : k * P + 1], 1.0)
            e_ring.append(t)

        c_tiles = {}
        stack_t = {}
        nm_t = {}
        ws_tiles = {}
        cnt_sq = [0]
        cnt_sub = [0]

        def p1_quad(g, q):
            # phase 1: err -> sq -> scan(nC) -> Pool partition-max into stack
            errq = pq.tile([128, QW], F32, tag="eq", name="err")
            for k in range(4):
                c = 4 * q + k
                sl = slice(k * P, (k + 1) * P)
                nc.tensor.matmul(
                    errq[:, sl],
                    lhsT=WHL[:, c * 128 : (c + 1) * 128],
                    rhs=XHH[:, g * P : (g + 1) * P],
                    start=True, stop=False, skip_group_check=True,
                )
                nc.tensor.matmul(
                    errq[:, sl],
                    lhsT=WH2[:, c * 128 : (c + 1) * 128],
                    rhs=XL2[:, g * P : (g + 1) * P],
                    start=False, stop=True, skip_group_check=True,
                )
            sq = sqp.tile([128, QW], F32, tag="sq", name="sq")
            j = cnt_sq[0]
            cnt_sq[0] += 1
            if (j * SQ_DMA) // 32 != ((j + 1) * SQ_DMA) // 32:
                errS = sqp.tile([128, QW], F32, tag="errS", name="errS", bufs=2)
                nc.sync.dma_start(errS[:], errq[:])
                nc.vector.tensor_tensor(sq[:], errS[:], errS[:], op=ALU.mult)
            else:
                nc.scalar.activation(sq[:], errq[:], AF.Square, bias=0.0, scale=1.0)

            nC = cp.tile([128, QW], F32, tag="c", name="c")
            c_tiles[(g, q)] = nC
            amg = AMg[g][:]
            for s in range(4):
                nc.vector.tensor_tensor_scan(
                    nC[:, s * P : (s + 1) * P], amg, sq[:, s * P : (s + 1) * P],
                    0.0, op0=ALU.add, op1=ALU.subtract,
                )
            if DBG and g == 0 and q == 0:
                nc.sync.dma_start(dbg_nc, nC[:])
            # per-quad partition max (over the 128 tasks of each chunk row);
            # gpsimd C-reduce must write partition 0, so bounce via DMA into
            # the per-group stack row
            stk = stack_t[g]
            ctmp = stp.tile([1, QW], F32, tag="ctmp", name="ctmp", bufs=3)
            with tc.high_priority(PRIO_MIN):
                nc.gpsimd.tensor_reduce(ctmp[:], nC[:], axis=AX.C, op=ALU.max)
                nc.sync.dma_start(stk[q : q + 1, :], ctmp[:])

        def p15(g, prio=None):
            # cross-quad max, fold 4 chunk-sections, broadcast -m
            ctx15 = tc.high_priority(PRIO_MIN if prio is None else prio)
            ctx15.__enter__()
            stk = stack_t[g]
            stk2 = stp.tile([NQ, QW], F32, tag="stk2", name=f"stk2_{g}")
            nc.gpsimd.partition_all_reduce(
                stk2[:], stk[:], channels=NQ, reduce_op=bass_isa.ReduceOp.max
            )
            sv = stk2[0:1, :].rearrange("p (s x) -> p s x", x=P)
            f01 = sm.tile([1, P], F32, tag="f01", name="f01")
            f23 = sm.tile([1, P], F32, tag="f23", name="f23")
            nc.vector.tensor_tensor(f01[:], sv[:, 0, :], sv[:, 1, :], op=ALU.max)
            nc.vector.tensor_tensor(f23[:], sv[:, 2, :], sv[:, 3, :], op=ALU.max)
            nm1 = sm.tile([1, P], F32, tag="nm1", name="nm1")
            nc.vector.tensor_tensor(nm1[:], f01[:], f23[:], op=ALU.max)
            nmB = sm.tile([128, P], F32, tag=f"nmB{g % 2}", name=f"nmB{g}")
            nc.gpsimd.partition_broadcast(nmB[:], nm1[:], channels=128)
            mh = None
            if SUB_PE > 0:
                mh = sm.tile([1, P], F32, tag=f"mh{g % 2}", name=f"mh{g}")
                nc.vector.tensor_scalar_mul(mh[:], nm1[:], -1.0)
            nm_t[g] = (nmB, mh)
            ctx15.__exit__(None, None, None)
            if DBG and g == 0:
                nc.sync.dma_start(dbg_stk, stk2[:])
                nc.sync.dma_start(dbg_nmb, nmB[:])
            wsb = wsp.tile([128, 2 * DA], F32, tag="wsj", name=f"ws{g}")
            ws_tiles[g] = wsb

        def p2_quad(g, q):
            # phase 2: cs = nC - (-m) = m - C; e = exp(cs) shifted; ws accum
            nC = c_tiles.pop((g, q))
            nmB, mh = nm_t[g]
            j = cnt_sub[0]
            cnt_sub[0] += 1
            on_pe = (j * SUB_PE) // 32 != ((j + 1) * SUB_PE) // 32
            if on_pe:
                cs = pq.tile([128, QW], F32, tag="eq", name="cs_ps")
                Cv = nC[:].rearrange("p (s x) -> p s x", x=P)
                for h in range(2):
                    sl = slice(h * 512, (h + 1) * 512)
                    nc.tensor.matmul(
                        cs[:, sl], lhsT=ID[:], rhs=Cv[:, 2 * h : 2 * h + 2, :],
                        start=True, stop=False, skip_group_check=True,
                    )
                    for hh in range(2):
                        nc.tensor.matmul(
                            cs[:, (2 * h + hh) * P : (2 * h + hh + 1) * P],
                            lhsT=ONR[:], rhs=mh[:],
                            start=False, stop=(hh == 1), skip_group_check=True,
                        )
            else:
                cs = csp.tile([128, QW], F32, tag="cs", name="cs")
                csv = cs[:].rearrange("p (s x) -> p s x", x=P)
                Cv = nC[:].rearrange("p (s x) -> p s x", x=P)
                nmv = (nmB[:].rearrange("p (a x) -> p a x", a=1)
                       .broadcast_to([128, 4, P]))
                nc.vector.tensor_tensor(csv, Cv, nmv, op=ALU.subtract)
            e = e_ring[(g * NQ + q) % len(e_ring)]
            ev = e[:].rearrange("p (s x) -> p s x", x=P)[:, :, 1:P]
            csv2 = cs[:].rearrange("p (s x) -> p s x", x=P)[:, :, 0 : P - 1]
            nc.scalar.activation(ev, csv2, AF.Exp, bias=0.0, scale=1.0)
            if DBG and g == 0:
                nc.sync.dma_start(dbg_e[q], e[:])
            wsb = ws_tiles[g]
            for k in range(4):
                c = 4 * q + k
                for j in range(2):
                    nc.tensor.matmul(
                        wsb[:, j * DA : (j + 1) * DA],
                        lhsT=e[:, k * P + j * 128 : k * P + (j + 1) * 128],
                        rhs=TP_sb[:, c, :],
                        start=(c == 0 and j == 0), stop=(c == NCH - 1),
                        skip_group_check=True,
                    )

        def p3(g):
            # out(q-lane, j) = (x . wsj[q, 0:64]) / wsj[q, 64]
            wsb = ws_tiles.pop(g)
            if DBG and g == 0:
                ws_dbg = sm.tile([128, 2 * DA], F32, tag="wsdbg", name="wsdbg")
                nc.vector.tensor_copy(ws_dbg[:], wsb[:])
                nc.sync.dma_start(dbg_ws, ws_dbg[:])
            nrg = sm.tile([128, 2], F32, tag="nrg", name="nrg")
            dcol = sm.tile([128, 2], F32, tag="dcol", name="dcol")
            for j in range(2):
                prod = sm.tile([128, D], F32, tag="prod", name="prod")
                nc.vector.tensor_tensor(
                    prod[:], XN[:, g * 2 + j, :], wsb[:, j * DA : j * DA + D],
                    op=ALU.mult,
                )
                nc.vector.tensor_reduce(
                    nrg[:, j : j + 1], prod[:], axis=AX.X, op=ALU.add
                )
                nc.vector.tensor_copy(
                    dcol[:, j : j + 1], wsb[:, j * DA + D : j * DA + D + 1]
                )
            rden = sm.tile([128, 2], F32, tag="rden", name="rden")
            nc.vector.reciprocal(rden[:], dcol[:])
            o = sm.tile([128, 2], F32, tag="o", name="o")
            nc.vector.tensor_tensor(o[:], nrg[:], rden[:], op=ALU.mult)
            nc.sync.dma_start(
                out_dram[g : g + 1, :].rearrange("b (h q) -> q (b h)", q=128),
                o[:],
            )

        # software-pipelined emission: interleave group g's phase-1 with
        # group g-1's phase-2; min-finalize (p15) hides behind early quads
        for g in range(BLOC):
            stack_t[g] = stp.tile([NQ, QW], F32, tag="stk", name=f"stk{g}")
            for q in range(NQ):
                p1_quad(g, q)
                if g == 0 and q == 2:
                    emit_late_dmas()
                if q == 0 and g > 0:
                    p15(g - 1)
                if g > 0 and q >= EARLY:
                    p2_quad(g - 1, q - EARLY)
            if g > 0:
                for q in range(NQ - EARLY, NQ):
                    p2_quad(g - 1, q)
                p3(g - 1)
        p15(BLOC - 1)
        for q in range(NQ):
            p2_quad(BLOC - 1, q)
        p3(BLOC - 1)


_CACHE = {}


def _get_nc():
    if "nc" not in _CACHE:
        nc = bacc.Bacc(
            "TRN2",
            target_bir_lowering=False,
            debug=False,
            enable_asserts=False,
            num_devices=NCORES,
        )
        with tile.TileContext(nc) as tc:
            build_program(tc)
        nc.compile()
        _CACHE["nc"] = nc
    return _CACHE["nc"]


def _split_pair(a):
    hi = a.astype(np.float16)
    lo = (a - hi.astype(np.float32)).astype(np.float16)
    return hi, lo


def _make_in_maps(data, targets, task_pool):
    data = np.ascontiguousarray(data, dtype=np.float32)
    targets = np.ascontiguousarray(targets, dtype=np.float32)
    task_pool = np.ascontiguousarray(task_pool, dtype=np.float32)
    isq2 = np.float32(1.0 / np.sqrt(2.0))
    W = task_pool[:, :, 0]  # (T, D)
    Ws = W.T * isq2  # (D, T), pre-scaled so sq = err^2 directly
    wh, wl = _split_pair(Ws)
    whl = np.concatenate([wh, wl], axis=0)  # (128, T) fp16
    wh2 = np.concatenate(
        [wh, np.ones((2, T), np.float16)], axis=0
    )  # (66, T): [Wh; 1; 1]
    tp32 = np.concatenate(
        [W, np.ones((T, 1), np.float32)], axis=1
    )  # (T, 65) fp32
    tp_pack = np.ascontiguousarray(
        tp32.reshape(NCH, 128, DA).transpose(1, 0, 2).reshape(128, -1)
    )
    ident = np.eye(128, dtype=np.float32)
    Wsub = W[::16]  # (256, D) deterministic subsample for lambda estimate
    in_maps = []
    for core in range(NCORES):
        xs = np.empty((D, BLOC * P), np.float32)
        ys = np.empty((BLOC * P,), np.float32)
        for j in range(BLOC):
            b = core * BLOC + j
            xs[:, j * P : (j + 1) * P] = data[b].T
            ys[j * P : (j + 1) * P] = targets[b]
        xh, xl = _split_pair(xs)
        xhh = np.concatenate([xh, xh], axis=0)  # (128, 1024)
        nys = -ys * isq2
        nyh, nyl = _split_pair(nys)
        xl2 = np.concatenate(
            [xl, nyh[None, :], nyl[None, :]], axis=0
        )  # (66, 1024): [xl; -yh; -yl]
        xn = np.ascontiguousarray(
            data[core * BLOC : (core + 1) * BLOC].reshape(BLOC * P, D)
        )
        xn_pack = np.ascontiguousarray(
            xn.reshape(2 * BLOC, 128, D).transpose(1, 0, 2).reshape(128, -1)
        )
        av = 0.5 * ((xn ** 2).sum(axis=1) + ys ** 2).astype(np.float32)
        # winner-targeted rebase: scale a by lambda ~= C_min/A (per batch,
        # estimated from a task subsample) so the scan state stays small for
        # the low-C tasks that dominate the posterior -> ~5x less fp32
        # rounding noise where it matters
        for j in range(BLOC):
            b = core * BLOC + j
            es = Wsub @ data[b].T - targets[b][None, :]
            Cs = 0.5 * (es ** 2).sum(axis=1)
            Ab = av[j * P : (j + 1) * P].sum()
            lam = np.float32(Cs.min() / (2.0 * Ab))
            av[j * P : (j + 1) * P] *= 2.0 * lam
        amask = np.broadcast_to(av[None, :], (128, BLOC * P)).copy()
        in_maps.append(
            {"whl": whl, "wh2": wh2, "xhh": xhh, "xl2": xl2, "tp32": tp_pack,
             "x_nat": xn_pack, "amask": amask, "ident": ident}
        )
    return in_maps


def run(data, targets, task_pool, trace=False):
    nc = _get_nc()
    in_maps = _make_in_maps(data, targets, task_pool)
    res = bass_utils.run_bass_kernel_spmd(
        nc, in_maps, core_ids=list(range(NCORES)), trace=trace
    )
    out = np.empty((B, P), np.float32)
    for core in range(NCORES):
        out[core * BLOC : (core + 1) * BLOC] = res.results[core]["out"]
    return out, res


def kernel(data, targets, task_pool):
    out, _ = run(data, targets, task_pool)
    return out


# revision 42
# speedup vs baseline: 1.3372x; 1.0322x over previous
"""DiscreteMMSE Trainium2 kernel (v12).

Math (per batch b, sharded 4 batches/core over 8 cores):
  W = task_pool[:,:,0]                        # (T, D)
  err  = (W@x - y)/sqrt(2)   (PE fp16 hi/lo: [Wh;Wl]@[xh;xh] K=128 +
                              [Wh;1;1]@[xl;-yh;-yl] K=66; residual ~2^-22;
                              W,y pre-scaled by 1/sqrt(2) on host)
  sq   = err^2               (ACT Square, PSUM in -> SBUF out)
  nC   = cumsum_p (a - sq)   (DVE tensor_tensor_scan per chunk section:
                              state=(a+state)-sq; a(j)=(|x_j|^2+y_j^2)/2 is a
                              per-point rebase that cancels in the softmax but
                              keeps the fp32 scan state ~5x smaller = ~5x less
                              rounding noise than the reference's own cumsum)
  -m(p)= max_t nC(t,p)       (Pool: gpsimd.tensor_reduce(axis=C) per quad
                              into rows of a per-group stack tile, then
                              partition_all_reduce(max) across quads, then
                              3 tiny DVE folds over the 4 chunk sections and
                              gpsimd.partition_broadcast -> nmB (128,P))
  cs   = nC - (-m) = m - C   (DVE TT subtract, or PE fp32 ident + fp16
                              rank-1 (+m) into PSUM, by knob)
  e    = exp(+cs) fp16       (ACT, shifted: e[:,s,1:256]=exp(cs[:,s,0:255]),
                              col 0 preset to 1 == uniform posterior at p=0)
  ws   = sum_t e(t,p)*[w_t|1]  (PE fp16: TP (128,65) stationary, e moving,
                              (65,256) PSUM accum over 32 chunks)
  out(p) = (x_p . ws[0:64,p]) / ws[64,p]  (TT prod + ones-matmul + recip)
The shift by m cancels exactly in the num/den ratio; cs <= 0 so exp never
overflows and den >= 1.

Sharding: data-parallel over batch: 32 batches -> 8 cores x 4. No collectives.
"""

import os
import sys

sys.path.insert(0, "/opt/trn_rl_repo")
sys.path.insert(0, "/opt/trn_rl_repo/concourse")

import numpy as np

import concourse.bass as bass
import concourse.tile as tile
from concourse import bacc, bass_isa, bass_utils, mybir

F32 = mybir.dt.float32
F16 = mybir.dt.float16
AF = mybir.ActivationFunctionType
ALU = mybir.AluOpType
AX = mybir.AxisListType

B, P, D, T = 32, 256, 64, 4096
NCORES = 8
BLOC = B // NCORES          # 4 batches per core = 4 groups
NCH = T // 128              # 32 task chunks
NQ = NCH // 4               # 8 quads (4 chunks each) per group
QW = 4 * P                  # quad tile width (1024)
DA = D + 1

# tuning knobs
SUB_PE = int(os.environ.get("KSBP", "7"))    # quads (of 32) subtracted on PE
EARLY = int(os.environ.get("KEARLY", "3"))
PRIO_MIN = int(os.environ.get("KPRIO", "40"))
DBG = int(os.environ.get("KDBG", "0"))        # dump group-0 intermediates


def build_program(tc):
    nc = tc.nc

    whl_dram = nc.dram_tensor("whl", (128, T), F16, kind="ExternalInput").ap()
    wh2_dram = nc.dram_tensor("wh2", (66, T), F16, kind="ExternalInput").ap()
    xhh_dram = nc.dram_tensor("xhh", (128, BLOC * P), F16, kind="ExternalInput").ap()
    xl2_dram = nc.dram_tensor("xl2", (66, BLOC * P), F16, kind="ExternalInput").ap()
    xn_dram = nc.dram_tensor("x_nat", (128, 2 * BLOC * D), F32, kind="ExternalInput").ap()
    am_dram = nc.dram_tensor("amask", (128, BLOC * P), F32, kind="ExternalInput").ap()
    tp_dram = nc.dram_tensor("tp32", (128, NCH * DA), F32, kind="ExternalInput").ap()
    id_dram = nc.dram_tensor("ident", (128, 128), F32, kind="ExternalInput").ap()
    out_dram = nc.dram_tensor("out", (BLOC, P), F32, kind="ExternalOutput").ap()
    if DBG:
        dbg_nc = nc.dram_tensor("dbg_nc", (128, QW), F32, kind="ExternalOutput").ap()
        dbg_stk = nc.dram_tensor("dbg_stk", (NQ, QW), F32, kind="ExternalOutput").ap()
        dbg_nmb = nc.dram_tensor("dbg_nmb", (128, P), F32, kind="ExternalOutput").ap()
        dbg_e = nc.dram_tensor("dbg_e", (NQ, 128, QW), F32, kind="ExternalOutput").ap()
        dbg_ws = nc.dram_tensor("dbg_ws", (128, 2 * DA), F32, kind="ExternalOutput").ap()

    from contextlib import ExitStack

    with ExitStack() as ctx:
        consts = ctx.enter_context(tc.tile_pool(name="consts", bufs=1))
        sqp = ctx.enter_context(tc.tile_pool(name="sqp", bufs=3))
        cp = ctx.enter_context(tc.tile_pool(name="cp", bufs=12))
        stp = ctx.enter_context(tc.tile_pool(name="stp", bufs=2))
        csp = ctx.enter_context(tc.tile_pool(name="csp", bufs=3))
        ep = ctx.enter_context(tc.tile_pool(name="ep", bufs=3))
        sm = ctx.enter_context(tc.tile_pool(name="sm", bufs=2))
        pq = ctx.enter_context(tc.tile_pool(name="pq", bufs=3, space="PSUM"))
        wsp = ctx.enter_context(tc.tile_pool(name="wsp", bufs=2, space="PSUM"))

        # ---- constants / inputs ----
        WHL = consts.tile([128, T], F16, tag="whl", name="whl")
        WH2 = consts.tile([66, T], F16, tag="wh2", name="wh2")
        XHH = consts.tile([128, BLOC * P], F16, tag="xhh", name="xhh")
        XL2 = consts.tile([66, BLOC * P], F16, tag="xl2", name="xl2")
        TP_sb = consts.tile([128, NCH, DA], F32, tag="tpsb", name="tpsb")
        XN = consts.tile([128, 2 * BLOC, D], F32, tag="xn", name="xn")
        AMg = [consts.tile([128, P], F32, tag=f"am{g}", name=f"am{g}")
               for g in range(BLOC)]
        ID = consts.tile([128, 128], F32, tag="ident", name="ident")
        ONR = consts.tile([1, 128], F32, tag="onr", name="onr")


        # ordered by first use: quad (g0,q0) needs XHH/XL2/WHL0/WH20, the
        # first scan needs AM; bulk/late tensors go via the ACT DGE queue so
        # the SP sequencer is free for the per-quad stack-bounce DMAs
        # HWDGE and the DMA engines serialize in acquire order, so order the
        # DMAs by first use: mm1(g0,q0) needs XHH+WHL cols 0:1024, mm2 needs
        # XL2+WH2, the first scan needs AM[g0]
        nc.sync.dma_start(XHH[:], xhh_dram)
        nc.sync.dma_start(WHL[:, 0:1024], whl_dram[:, 0:1024])
        nc.sync.dma_start(XL2[:], xl2_dram)
        nc.sync.dma_start(WH2[:, 0:1024], wh2_dram[:, 0:1024])
        nc.sync.dma_start(AMg[0][:], am_dram[:, 0:P])
        nc.sync.dma_start(WHL[:, 1024:], whl_dram[:, 1024:])
        nc.sync.dma_start(WH2[:, 1024:], wh2_dram[:, 1024:])
        for g in range(1, BLOC):
            nc.sync.dma_start(AMg[g][:], am_dram[:, g * P : (g + 1) * P])
        nc.gpsimd.memset(ONR[:], 1.0)
        late_dma = [False]

        def emit_late_dmas():
            # ID/XN/TP are first needed at p15(0)/p2(0)/p3(0); emitting them
            # here (mid group 0) keeps the ACT sequencer free for the first
            # squares at startup
            if late_dma[0]:
                return
            late_dma[0] = True
            with tc.tile_wait_until(0.012):
                nc.scalar.dma_start(ID[:], id_dram)
                nc.scalar.dma_start(
                    XN[:], xn_dram.rearrange("q (j d) -> q j d", d=D))
                nc.scalar.dma_start(
                    TP_sb[:], tp_dram.rearrange("q (c d) -> q c d", d=DA))

        # e ring: fp16, col 0 of each 256-section preset to 1.0 (p=0 uniform)
        e_ring = []
        for i in range(3):
            t = ep.tile([128, QW], F32, tag=f"e{i}", name=f"e{i}")
            for k in range(4):
                nc.gpsimd.memset(t[:, k * P : k * P + 1], 1.0)
            e_ring.append(t)

        c_tiles = {}
        stack_t = {}
        nm_t = {}
        ws_tiles = {}
        cnt_sub = [0]

        def p1_quad(g, q):
            # phase 1: err -> sq -> scan(nC) -> Pool partition-max into stack
            errq = pq.tile([128, QW], F32, tag="eq", name="err")
            for k in range(4):
                c = 4 * q + k
                sl = slice(k * P, (k + 1) * P)
                nc.tensor.matmul(
                    errq[:, sl],
                    lhsT=WHL[:, c * 128 : (c + 1) * 128],
                    rhs=XHH[:, g * P : (g + 1) * P],
                    start=True, stop=False, skip_group_check=True,
                )
                nc.tensor.matmul(
                    errq[:, sl],
                    lhsT=WH2[:, c * 128 : (c + 1) * 128],
                    rhs=XL2[:, g * P : (g + 1) * P],
                    start=False, stop=True, skip_group_check=True,
                )
            sq = sqp.tile([128, QW], F32, tag="sq", name="sq")
            nc.scalar.activation(sq[:], errq[:], AF.Square, bias=0.0, scale=1.0)

            nC = cp.tile([128, QW], F32, tag="c", name="c")
            c_tiles[(g, q)] = nC
            amg = AMg[g][:]
            for s in range(4):
                nc.vector.tensor_tensor_scan(
                    nC[:, s * P : (s + 1) * P], amg, sq[:, s * P : (s + 1) * P],
                    0.0, op0=ALU.add, op1=ALU.subtract,
                )
            if DBG and g == 0 and q == 0:
                nc.sync.dma_start(dbg_nc, nC[:])
            # per-quad partition max (over the 128 tasks of each chunk row);
            # gpsimd C-reduce must write partition 0, so bounce via DMA into
            # the per-group stack row
            stk = stack_t[g]
            ctmp = stp.tile([1, QW], F32, tag="ctmp", name="ctmp", bufs=3)
            with tc.high_priority(PRIO_MIN):
                nc.gpsimd.tensor_reduce(ctmp[:], nC[:], axis=AX.C, op=ALU.max)
                nc.sync.dma_start(stk[4 * q : 4 * (q + 1), :], ctmp[:])

        def p15(g, prio=None):
            # cross-quad max, fold 4 chunk-sections, broadcast -m
            ctx15 = tc.high_priority(PRIO_MIN if prio is None else prio)
            ctx15.__enter__()
            stk = stack_t[g]
            stk2 = stp.tile([4 * NQ, P], F32, tag="stk2", name=f"stk2_{g}")
            nc.gpsimd.partition_all_reduce(
                stk2[:], stk[:], channels=4 * NQ, reduce_op=bass_isa.ReduceOp.max
            )
            nmB = sm.tile([128, P], F32, tag=f"nmB{g % 2}", name=f"nmB{g}")
            nc.gpsimd.partition_broadcast(nmB[:], stk2[0:1, :], channels=128)
            mh = None
            if SUB_PE > 0:
                mh = sm.tile([1, P], F32, tag=f"mh{g % 2}", name=f"mh{g}")
                nc.vector.tensor_scalar_mul(mh[:], stk2[0:1, :], -1.0)
            nm_t[g] = (nmB, mh)
            ctx15.__exit__(None, None, None)
            if DBG and g == 0:
                nc.sync.dma_start(dbg_stk, stk2[:])
                nc.sync.dma_start(dbg_nmb, nmB[:])
            wsb = wsp.tile([128, 2 * DA], F32, tag="wsj", name=f"ws{g}")
            ws_tiles[g] = wsb

        def p2_quad(g, q):
            # phase 2: cs = nC - (-m) = m - C; e = exp(cs) shifted; ws accum
            nC = c_tiles.pop((g, q))
            nmB, mh = nm_t[g]
            j = cnt_sub[0]
            cnt_sub[0] += 1
            on_pe = (j * SUB_PE) // 32 != ((j + 1) * SUB_PE) // 32
            if on_pe:
                cs = pq.tile([128, QW], F32, tag="eq", name="cs_ps")
                Cv = nC[:].rearrange("p (s x) -> p s x", x=P)
                for h in range(2):
                    sl = slice(h * 512, (h + 1) * 512)
                    nc.tensor.matmul(
                        cs[:, sl], lhsT=ID[:], rhs=Cv[:, 2 * h : 2 * h + 2, :],
                        start=True, stop=False, skip_group_check=True,
                    )
                    for hh in range(2):
                        nc.tensor.matmul(
                            cs[:, (2 * h + hh) * P : (2 * h + hh + 1) * P],
                            lhsT=ONR[:], rhs=mh[:],
                            start=False, stop=(hh == 1), skip_group_check=True,
                        )
            else:
                cs = csp.tile([128, QW], F32, tag="cs", name="cs")
                csv = cs[:].rearrange("p (s x) -> p s x", x=P)
                Cv = nC[:].rearrange("p (s x) -> p s x", x=P)
                nmv = (nmB[:].rearrange("p (a x) -> p a x", a=1)
                       .broadcast_to([128, 4, P]))
                nc.vector.tensor_tensor(csv, Cv, nmv, op=ALU.subtract)
            e = e_ring[(g * NQ + q) % len(e_ring)]
            ev = e[:].rearrange("p (s x) -> p s x", x=P)[:, :, 1:P]
            csv2 = cs[:].rearrange("p (s x) -> p s x", x=P)[:, :, 0 : P - 1]
            nc.scalar.activation(ev, csv2, AF.Exp, bias=0.0, scale=1.0)
            if DBG and g == 0:
                nc.sync.dma_start(dbg_e[q], e[:])
            wsb = ws_tiles[g]
            for k in range(4):
                c = 4 * q + k
                for j in range(2):
                    nc.tensor.matmul(
                        wsb[:, j * DA : (j + 1) * DA],
                        lhsT=e[:, k * P + j * 128 : k * P + (j + 1) * 128],
                        rhs=TP_sb[:, c, :],
                        start=(c == 0 and j == 0), stop=(c == NCH - 1),
                        skip_group_check=True,
                    )

        def p3(g):
            # out(q-lane, j) = (x . wsj[q, 0:64]) / wsj[q, 64]
            wsb = ws_tiles.pop(g)
            if DBG and g == 0:
                ws_dbg = sm.tile([128, 2 * DA], F32, tag="wsdbg", name="wsdbg")
                nc.vector.tensor_copy(ws_dbg[:], wsb[:])
                nc.sync.dma_start(dbg_ws, ws_dbg[:])
            nrg = sm.tile([128, 2], F32, tag="nrg", name="nrg")
            dcol = sm.tile([128, 2], F32, tag="dcol", name="dcol")
            for j in range(2):
                prod = sm.tile([128, D], F32, tag="prod", name="prod")
                nc.vector.tensor_tensor(
                    prod[:], XN[:, g * 2 + j, :], wsb[:, j * DA : j * DA + D],
                    op=ALU.mult,
                )
                nc.vector.tensor_reduce(
                    nrg[:, j : j + 1], prod[:], axis=AX.X, op=ALU.add
                )
                nc.vector.tensor_copy(
                    dcol[:, j : j + 1], wsb[:, j * DA + D : j * DA + D + 1]
                )
            rden = sm.tile([128, 2], F32, tag="rden", name="rden")
            nc.vector.reciprocal(rden[:], dcol[:])
            o = sm.tile([128, 2], F32, tag="o", name="o")
            nc.vector.tensor_tensor(o[:], nrg[:], rden[:], op=ALU.mult)
            nc.sync.dma_start(
                out_dram[g : g + 1, :].rearrange("b (h q) -> q (b h)", q=128),
                o[:],
            )

        # software-pipelined emission: interleave group g's phase-1 with
        # group g-1's phase-2; min-finalize (p15) hides behind early quads
        for g in range(BLOC):
            stack_t[g] = stp.tile([4 * NQ, P], F32, tag="stk", name=f"stk{g}")
            for q in range(NQ):
                p1_quad(g, q)
                if g == 0 and q == 2:
                    emit_late_dmas()
                if q == 0 and g > 0:
                    p15(g - 1)
                if g > 0 and q >= EARLY:
                    p2_quad(g - 1, q - EARLY)
            if g > 0:
                for q in range(NQ - EARLY, NQ):
                    p2_quad(g - 1, q)
                p3(g - 1)
        p15(BLOC - 1)
        for q in range(NQ):
            p2_quad(BLOC - 1, q)
        p3(BLOC - 1)


_CACHE = {}


def _get_nc():
    if "nc" not in _CACHE:
        nc = bacc.Bacc(
            "TRN2",
            target_bir_lowering=False,
            debug=False,
            enable_asserts=False,
            num_devices=NCORES,
        )
        with tile.TileContext(nc) as tc:
            build_program(tc)
        nc.compile()
        _CACHE["nc"] = nc
    return _CACHE["nc"]


def _split_pair(a):
    hi = a.astype(np.float16)
    lo = (a - hi.astype(np.float32)).astype(np.float16)
    return hi, lo


def _make_in_maps(data, targets, task_pool):
    data = np.ascontiguousarray(data, dtype=np.float32)
    targets = np.ascontiguousarray(targets, dtype=np.float32)
    task_pool = np.ascontiguousarray(task_pool, dtype=np.float32)
    isq2 = np.float32(1.0 / np.sqrt(2.0))
    W = task_pool[:, :, 0]  # (T, D)
    Ws = W.T * isq2  # (D, T), pre-scaled so sq = err^2 directly
    wh, wl = _split_pair(Ws)
    whl = np.concatenate([wh, wl], axis=0)  # (128, T) fp16
    wh2 = np.concatenate(
        [wh, np.ones((2, T), np.float16)], axis=0
    )  # (66, T): [Wh; 1; 1]
    tp32 = np.concatenate(
        [W, np.ones((T, 1), np.float32)], axis=1
    )  # (T, 65) fp32
    tp_pack = np.ascontiguousarray(
        tp32.reshape(NCH, 128, DA).transpose(1, 0, 2).reshape(128, -1)
    )
    ident = np.eye(128, dtype=np.float32)
    Wsub = W[::16]  # (256, D) deterministic subsample for lambda estimate
    in_maps = []
    for core in range(NCORES):
        xs = np.empty((D, BLOC * P), np.float32)
        ys = np.empty((BLOC * P,), np.float32)
        for j in range(BLOC):
            b = core * BLOC + j
            xs[:, j * P : (j + 1) * P] = data[b].T
            ys[j * P : (j + 1) * P] = targets[b]
        xh, xl = _split_pair(xs)
        xhh = np.concatenate([xh, xh], axis=0)  # (128, 1024)
        nys = -ys * isq2
        nyh, nyl = _split_pair(nys)
        xl2 = np.concatenate(
            [xl, nyh[None, :], nyl[None, :]], axis=0
        )  # (66, 1024): [xl; -yh; -yl]
        xn = np.ascontiguousarray(
            data[core * BLOC : (core + 1) * BLOC].reshape(BLOC * P, D)
        )
        xn_pack = np.ascontiguousarray(
            xn.reshape(2 * BLOC, 128, D).transpose(1, 0, 2).reshape(128, -1)
        )
        av = 0.5 * ((xn ** 2).sum(axis=1) + ys ** 2).astype(np.float32)
        # winner-targeted rebase: scale a by lambda ~= C_min/A (per batch,
        # estimated from a task subsample) so the scan state stays small for
        # the low-C tasks that dominate the posterior -> ~5x less fp32
        # rounding noise where it matters
        for j in range(BLOC):
            b = core * BLOC + j
            es = Wsub @ data[b].T - targets[b][None, :]
            Cs = 0.5 * (es ** 2).sum(axis=1)
            Ab = av[j * P : (j + 1) * P].sum()
            lam = np.float32(Cs.min() / (2.0 * Ab))
            av[j * P : (j + 1) * P] *= 2.0 * lam
        amask = np.broadcast_to(av[None, :], (128, BLOC * P)).copy()
        in_maps.append(
            {"whl": whl, "wh2": wh2, "xhh": xhh, "xl2": xl2, "tp32": tp_pack,
             "x_nat": xn_pack, "amask": amask, "ident": ident}
        )
    return in_maps


def run(data, targets, task_pool, trace=False):
    nc = _get_nc()
    in_maps = _make_in_maps(data, targets, task_pool)
    res = bass_utils.run_bass_kernel_spmd(
        nc, in_maps, core_ids=list(range(NCORES)), trace=trace
    )
    out = np.empty((B, P), np.float32)
    for core in range(NCORES):
        out[core * BLOC : (core + 1) * BLOC] = res.results[core]["out"]
    return out, res


def kernel(data, targets, task_pool):
    out, _ = run(data, targets, task_pool)
    return out


# revision 47
# speedup vs baseline: 1.3465x; 1.0069x over previous
"""DiscreteMMSE Trainium2 kernel (v12).

Math (per batch b, sharded 4 batches/core over 8 cores):
  W = task_pool[:,:,0]                        # (T, D)
  err  = (W@x - y)/sqrt(2)   (PE fp16 hi/lo: [Wh;Wl]@[xh;xh] K=128 +
                              [Wh;1;1]@[xl;-yh;-yl] K=66; residual ~2^-22;
                              W,y pre-scaled by 1/sqrt(2) on host)
  sq   = err^2               (ACT Square, PSUM in -> SBUF out)
  nC   = cumsum_p (a - sq)   (DVE tensor_tensor_scan per chunk section:
                              state=(a+state)-sq; a(j)=(|x_j|^2+y_j^2)/2 is a
                              per-point rebase that cancels in the softmax but
                              keeps the fp32 scan state ~5x smaller = ~5x less
                              rounding noise than the reference's own cumsum)
  -m(p)= max_t nC(t,p)       (Pool: gpsimd.tensor_reduce(axis=C) per quad
                              into rows of a per-group stack tile, then
                              partition_all_reduce(max) across quads, then
                              3 tiny DVE folds over the 4 chunk sections and
                              gpsimd.partition_broadcast -> nmB (128,P))
  cs   = nC - (-m) = m - C   (DVE TT subtract, or PE fp32 ident + fp16
                              rank-1 (+m) into PSUM, by knob)
  e    = exp(+cs) fp16       (ACT, shifted: e[:,s,1:256]=exp(cs[:,s,0:255]),
                              col 0 preset to 1 == uniform posterior at p=0)
  ws   = sum_t e(t,p)*[w_t|1]  (PE fp16: TP (128,65) stationary, e moving,
                              (65,256) PSUM accum over 32 chunks)
  out(p) = (x_p . ws[0:64,p]) / ws[64,p]  (TT prod + ones-matmul + recip)
The shift by m cancels exactly in the num/den ratio; cs <= 0 so exp never
overflows and den >= 1.

Sharding: data-parallel over batch: 32 batches -> 8 cores x 4. No collectives.
"""

import os
import sys

sys.path.insert(0, "/opt/trn_rl_repo")
sys.path.insert(0, "/opt/trn_rl_repo/concourse")

import numpy as np

import concourse.bass as bass
import concourse.tile as tile
from concourse import bacc, bass_isa, bass_utils, mybir

F32 = mybir.dt.float32
F16 = mybir.dt.float16
AF = mybir.ActivationFunctionType
ALU = mybir.AluOpType
AX = mybir.AxisListType

B, P, D, T = 32, 256, 64, 4096
NCORES = 8
BLOC = B // NCORES          # 4 batches per core = 4 groups
NCH = T // 128              # 32 task chunks
NQ = NCH // 4               # 8 quads (4 chunks each) per group
QW = 4 * P                  # quad tile width (1024)
DA = D + 1

# tuning knobs
SUB_PE = int(os.environ.get("KSBP", "7"))    # quads (of 32) subtracted on PE
SUB_POOL = int(os.environ.get("KSBL", "0"))   # of the rest, quads on Pool
B_CS = int(os.environ.get("KBCS", "3"))
B_E = int(os.environ.get("KBE", "3"))
B_SQ = int(os.environ.get("KBSQ", "3"))
B_CP = int(os.environ.get("KBCP", "12"))
EARLY = int(os.environ.get("KEARLY", "3"))
PRIO_MIN = int(os.environ.get("KPRIO", "40"))
DBG = int(os.environ.get("KDBG", "0"))        # dump group-0 intermediates


def build_program(tc):
    nc = tc.nc

    whl_dram = nc.dram_tensor("whl", (128, T), F16, kind="ExternalInput").ap()
    wh2_dram = nc.dram_tensor("wh2", (66, T), F16, kind="ExternalInput").ap()
    xhh_dram = nc.dram_tensor("xhh", (128, BLOC * P), F16, kind="ExternalInput").ap()
    xl2_dram = nc.dram_tensor("xl2", (66, BLOC * P), F16, kind="ExternalInput").ap()
    xn_dram = nc.dram_tensor("x_nat", (128, 2 * BLOC * D), F32, kind="ExternalInput").ap()
    am_dram = nc.dram_tensor("amask", (128, BLOC * P), F32, kind="ExternalInput").ap()
    tp_dram = nc.dram_tensor("tp32", (128, NCH * DA), F32, kind="ExternalInput").ap()
    id_dram = nc.dram_tensor("ident", (128, 128), F32, kind="ExternalInput").ap()
    out_dram = nc.dram_tensor("out", (BLOC, P), F32, kind="ExternalOutput").ap()
    if DBG:
        dbg_nc = nc.dram_tensor("dbg_nc", (128, QW), F32, kind="ExternalOutput").ap()
        dbg_stk = nc.dram_tensor("dbg_stk", (NQ, QW), F32, kind="ExternalOutput").ap()
        dbg_nmb = nc.dram_tensor("dbg_nmb", (128, P), F32, kind="ExternalOutput").ap()
        dbg_e = nc.dram_tensor("dbg_e", (NQ, 128, QW), F32, kind="ExternalOutput").ap()
        dbg_ws = nc.dram_tensor("dbg_ws", (128, 2 * DA), F32, kind="ExternalOutput").ap()

    from contextlib import ExitStack

    with ExitStack() as ctx:
        consts = ctx.enter_context(tc.tile_pool(name="consts", bufs=1))
        sqp = ctx.enter_context(tc.tile_pool(name="sqp", bufs=B_SQ))
        cp = ctx.enter_context(tc.tile_pool(name="cp", bufs=B_CP))
        stp = ctx.enter_context(tc.tile_pool(name="stp", bufs=2))
        csp = ctx.enter_context(tc.tile_pool(name="csp", bufs=B_CS))
        ep = ctx.enter_context(tc.tile_pool(name="ep", bufs=B_E))
        sm = ctx.enter_context(tc.tile_pool(name="sm", bufs=2))
        pq = ctx.enter_context(tc.tile_pool(name="pq", bufs=3, space="PSUM"))
        wsp = ctx.enter_context(tc.tile_pool(name="wsp", bufs=2, space="PSUM"))

        # ---- constants / inputs ----
        WHL = consts.tile([128, T], F16, tag="whl", name="whl")
        WH2 = consts.tile([66, T], F16, tag="wh2", name="wh2")
        XHH = consts.tile([128, BLOC * P], F16, tag="xhh", name="xhh")
        XL2 = consts.tile([66, BLOC * P], F16, tag="xl2", name="xl2")
        TP_sb = consts.tile([128, NCH, DA], F32, tag="tpsb", name="tpsb")
        XN = consts.tile([128, 2 * BLOC, D], F32, tag="xn", name="xn")
        AMg = [consts.tile([128, P], F32, tag=f"am{g}", name=f"am{g}")
               for g in range(BLOC)]
        ID = consts.tile([128, 128], F32, tag="ident", name="ident")
        ONR = consts.tile([1, 128], F32, tag="onr", name="onr")


        # ordered by first use: quad (g0,q0) needs XHH/XL2/WHL0/WH20, the
        # first scan needs AM; bulk/late tensors go via the ACT DGE queue so
        # the SP sequencer is free for the per-quad stack-bounce DMAs
        # HWDGE and the DMA engines serialize in acquire order, so order the
        # DMAs by first use: mm1(g0,q0) needs XHH+WHL cols 0:1024, mm2 needs
        # XL2+WH2, the first scan needs AM[g0]
        nc.sync.dma_start(XHH[:], xhh_dram)
        nc.sync.dma_start(WHL[:, 0:1024], whl_dram[:, 0:1024])
        nc.sync.dma_start(XL2[:], xl2_dram)
        nc.sync.dma_start(WH2[:, 0:1024], wh2_dram[:, 0:1024])
        nc.sync.dma_start(AMg[0][:], am_dram[:, 0:P])
        nc.sync.dma_start(WHL[:, 1024:], whl_dram[:, 1024:])
        nc.sync.dma_start(WH2[:, 1024:], wh2_dram[:, 1024:])
        for g in range(1, BLOC):
            nc.sync.dma_start(AMg[g][:], am_dram[:, g * P : (g + 1) * P])
        nc.gpsimd.memset(ONR[:], 1.0)
        late_dma = [False]

        def emit_late_dmas():
            # ID/XN/TP are first needed at p15(0)/p2(0)/p3(0); emitting them
            # here (mid group 0) keeps the ACT sequencer free for the first
            # squares at startup
            if late_dma[0]:
                return
            late_dma[0] = True
            with tc.tile_wait_until(0.012):
                nc.scalar.dma_start(ID[:], id_dram)
                nc.scalar.dma_start(
                    XN[:], xn_dram.rearrange("q (j d) -> q j d", d=D))
                nc.scalar.dma_start(
                    TP_sb[:], tp_dram.rearrange("q (c d) -> q c d", d=DA))

        # e ring: fp16, col 0 of each 256-section preset to 1.0 (p=0 uniform)
        e_ring = []
        for i in range(B_E):
            t = ep.tile([128, QW], F32, tag=f"e{i}", name=f"e{i}")
            for k in range(4):
                nc.gpsimd.memset(t[:, k * P : k * P + 1], 1.0)
            e_ring.append(t)

        c_tiles = {}
        stack_t = {}
        nm_t = {}
        ws_tiles = {}
        cnt_sub = [0]

        def p1_quad(g, q):
            # phase 1: err -> sq -> scan(nC) -> Pool partition-max into stack
            errq = pq.tile([128, QW], F32, tag="eq", name="err")
            for k in range(4):
                c = 4 * q + k
                sl = slice(k * P, (k + 1) * P)
                nc.tensor.matmul(
                    errq[:, sl],
                    lhsT=WHL[:, c * 128 : (c + 1) * 128],
                    rhs=XHH[:, g * P : (g + 1) * P],
                    start=True, stop=False, skip_group_check=True,
                )
                nc.tensor.matmul(
                    errq[:, sl],
                    lhsT=WH2[:, c * 128 : (c + 1) * 128],
                    rhs=XL2[:, g * P : (g + 1) * P],
                    start=False, stop=True, skip_group_check=True,
                )
            sq = sqp.tile([128, QW], F32, tag="sq", name="sq")
            nc.scalar.activation(sq[:], errq[:], AF.Square, bias=0.0, scale=1.0)

            nC = cp.tile([128, QW], F32, tag="c", name="c")
            c_tiles[(g, q)] = nC
            amg = AMg[g][:]
            from contextlib import nullcontext
            pctx = (tc.high_priority(PRIO_MIN) if g == BLOC - 1
                    else nullcontext())
            with pctx:
                for s in range(4):
                    nc.vector.tensor_tensor_scan(
                        nC[:, s * P : (s + 1) * P], amg,
                        sq[:, s * P : (s + 1) * P],
                        0.0, op0=ALU.add, op1=ALU.subtract,
                    )
            if DBG and g == 0 and q == 0:
                nc.sync.dma_start(dbg_nc, nC[:])
            # per-quad partition max (over the 128 tasks of each chunk row);
            # gpsimd C-reduce must write partition 0, so bounce via DMA into
            # the per-group stack row
            stk = stack_t[g]
            ctmp = stp.tile([1, QW], F32, tag="ctmp", name="ctmp", bufs=3)
            with tc.high_priority(PRIO_MIN):
                nc.gpsimd.tensor_reduce(ctmp[:], nC[:], axis=AX.C, op=ALU.max)
                nc.sync.dma_start(stk[4 * q : 4 * (q + 1), :], ctmp[:])

        def p15(g, prio=None):
            # cross-quad max, fold 4 chunk-sections, broadcast -m
            ctx15 = tc.high_priority(PRIO_MIN if prio is None else prio)
            ctx15.__enter__()
            stk = stack_t[g]
            stk2 = stp.tile([4 * NQ, P], F32, tag="stk2", name=f"stk2_{g}")
            nc.gpsimd.partition_all_reduce(
                stk2[:], stk[:], channels=4 * NQ, reduce_op=bass_isa.ReduceOp.max
            )
            nmB = sm.tile([128, P], F32, tag=f"nmB{g % 2}", name=f"nmB{g}")
            nc.gpsimd.partition_broadcast(nmB[:], stk2[0:1, :], channels=128)
            mh = None
            if SUB_PE > 0:
                mh = sm.tile([1, P], F32, tag=f"mh{g % 2}", name=f"mh{g}")
                nc.vector.tensor_scalar_mul(mh[:], stk2[0:1, :], -1.0)
            nm_t[g] = (nmB, mh)
            ctx15.__exit__(None, None, None)
            if DBG and g == 0:
                nc.sync.dma_start(dbg_stk, stk2[:])
                nc.sync.dma_start(dbg_nmb, nmB[:])
            wsb = wsp.tile([128, 2 * DA], F32, tag="wsj", name=f"ws{g}")
            ws_tiles[g] = wsb

        def p2_quad(g, q):
            # phase 2: cs = nC - (-m) = m - C; e = exp(cs) shifted; ws accum
            nC = c_tiles.pop((g, q))
            nmB, mh = nm_t[g]
            j = cnt_sub[0]
            cnt_sub[0] += 1
            on_pe = (j * SUB_PE) // 32 != ((j + 1) * SUB_PE) // 32
            on_pool = (not on_pe and
                       (j * SUB_POOL) // 32 != ((j + 1) * SUB_POOL) // 32)
            if on_pe:
                cs = pq.tile([128, QW], F32, tag="eq", name="cs_ps")
                Cv = nC[:].rearrange("p (s x) -> p s x", x=P)
                for h in range(2):
                    sl = slice(h * 512, (h + 1) * 512)
                    nc.tensor.matmul(
                        cs[:, sl], lhsT=ID[:], rhs=Cv[:, 2 * h : 2 * h + 2, :],
                        start=True, stop=False, skip_group_check=True,
                    )
                    for hh in range(2):
                        nc.tensor.matmul(
                            cs[:, (2 * h + hh) * P : (2 * h + hh + 1) * P],
                            lhsT=ONR[:], rhs=mh[:],
                            start=False, stop=(hh == 1), skip_group_check=True,
                        )
            else:
                cs = csp.tile([128, QW], F32, tag="cs", name="cs")
                csv = cs[:].rearrange("p (s x) -> p s x", x=P)
                Cv = nC[:].rearrange("p (s x) -> p s x", x=P)
                nmv = (nmB[:].rearrange("p (a x) -> p a x", a=1)
                       .broadcast_to([128, 4, P]))
                if on_pool:
                    nc.gpsimd.scalar_tensor_tensor(
                        csv, Cv, 1.0, nmv, op0=ALU.mult, op1=ALU.subtract
                    )
                else:
                    nc.vector.tensor_tensor(csv, Cv, nmv, op=ALU.subtract)
            e = e_ring[(g * NQ + q) % B_E]
            ev = e[:].rearrange("p (s x) -> p s x", x=P)[:, :, 1:P]
            csv2 = cs[:].rearrange("p (s x) -> p s x", x=P)[:, :, 0 : P - 1]
            nc.scalar.activation(ev, csv2, AF.Exp, bias=0.0, scale=1.0)
            if DBG and g == 0:
                nc.sync.dma_start(dbg_e[q], e[:])
            wsb = ws_tiles[g]
            for k in range(4):
                c = 4 * q + k
                for j in range(2):
                    nc.tensor.matmul(
                        wsb[:, j * DA : (j + 1) * DA],
                        lhsT=e[:, k * P + j * 128 : k * P + (j + 1) * 128],
                        rhs=TP_sb[:, c, :],
                        start=(c == 0 and j == 0), stop=(c == NCH - 1),
                        skip_group_check=True,
                    )

        def p3(g):
            # out(q-lane, j) = (x . wsj[q, 0:64]) / wsj[q, 64]
            wsb = ws_tiles.pop(g)
            if DBG and g == 0:
                ws_dbg = sm.tile([128, 2 * DA], F32, tag="wsdbg", name="wsdbg")
                nc.vector.tensor_copy(ws_dbg[:], wsb[:])
                nc.sync.dma_start(dbg_ws, ws_dbg[:])
            nrg = sm.tile([128, 2], F32, tag="nrg", name="nrg")
            dcol = sm.tile([128, 2], F32, tag="dcol", name="dcol")
            for j in range(2):
                prod = sm.tile([128, D], F32, tag="prod", name="prod")
                nc.vector.tensor_tensor(
                    prod[:], XN[:, g * 2 + j, :], wsb[:, j * DA : j * DA + D],
                    op=ALU.mult,
                )
                nc.vector.tensor_reduce(
                    nrg[:, j : j + 1], prod[:], axis=AX.X, op=ALU.add
                )
                nc.vector.tensor_copy(
                    dcol[:, j : j + 1], wsb[:, j * DA + D : j * DA + D + 1]
                )
            rden = sm.tile([128, 2], F32, tag="rden", name="rden")
            nc.vector.reciprocal(rden[:], dcol[:])
            o = sm.tile([128, 2], F32, tag="o", name="o")
            nc.vector.tensor_tensor(o[:], nrg[:], rden[:], op=ALU.mult)
            nc.sync.dma_start(
                out_dram[g : g + 1, :].rearrange("b (h q) -> q (b h)", q=128),
                o[:],
            )

        # software-pipelined emission: interleave group g's phase-1 with
        # group g-1's phase-2; min-finalize (p15) hides behind early quads
        for g in range(BLOC):
            stack_t[g] = stp.tile([4 * NQ, P], F32, tag="stk", name=f"stk{g}")
            for q in range(NQ):
                p1_quad(g, q)
                if g == 0 and q == 2:
                    emit_late_dmas()
                if q == 0 and g > 0:
                    p15(g - 1)
                if g > 0 and q >= EARLY:
                    p2_quad(g - 1, q - EARLY)
            if g > 0:
                for q in range(NQ - EARLY, NQ):
                    p2_quad(g - 1, q)
                p3(g - 1)
        p15(BLOC - 1)
        for q in range(NQ):
            p2_quad(BLOC - 1, q)
        p3(BLOC - 1)


_CACHE = {}


def _get_nc():
    if "nc" not in _CACHE:
        nc = bacc.Bacc(
            "TRN2",
            target_bir_lowering=False,
            debug=False,
            enable_asserts=False,
            num_devices=NCORES,
        )
        with tile.TileContext(nc) as tc:
            build_program(tc)
        nc.compile()
        _CACHE["nc"] = nc
    return _CACHE["nc"]


def _split_pair(a):
    hi = a.astype(np.float16)
    lo = (a - hi.astype(np.float32)).astype(np.float16)
    return hi, lo


def _make_in_maps(data, targets, task_pool):
    data = np.ascontiguousarray(data, dtype=np.float32)
    targets = np.ascontiguousarray(targets, dtype=np.float32)
    task_pool = np.ascontiguousarray(task_pool, dtype=np.float32)
    isq2 = np.float32(1.0 / np.sqrt(2.0))
    W = task_pool[:, :, 0]  # (T, D)
    Ws = W.T * isq2  # (D, T), pre-scaled so sq = err^2 directly
    wh, wl = _split_pair(Ws)
    whl = np.concatenate([wh, wl], axis=0)  # (128, T) fp16
    wh2 = np.concatenate(
        [wh, np.ones((2, T), np.float16)], axis=0
    )  # (66, T): [Wh; 1; 1]
    tp32 = np.concatenate(
        [W, np.ones((T, 1), np.float32)], axis=1
    )  # (T, 65) fp32
    tp_pack = np.ascontiguousarray(
        tp32.reshape(NCH, 128, DA).transpose(1, 0, 2).reshape(128, -1)
    )
    ident = np.eye(128, dtype=np.float32)
    Wsub = W[::16]  # (256, D) deterministic subsample for lambda estimate
    in_maps = []
    for core in range(NCORES):
        xs = np.empty((D, BLOC * P), np.float32)
        ys = np.empty((BLOC * P,), np.float32)
        for j in range(BLOC):
            b = core * BLOC + j
            xs[:, j * P : (j + 1) * P] = data[b].T
            ys[j * P : (j + 1) * P] = targets[b]
        xh, xl = _split_pair(xs)
        xhh = np.concatenate([xh, xh], axis=0)  # (128, 1024)
        nys = -ys * isq2
        nyh, nyl = _split_pair(nys)
        xl2 = np.concatenate(
            [xl, nyh[None, :], nyl[None, :]], axis=0
        )  # (66, 1024): [xl; -yh; -yl]
        xn = np.ascontiguousarray(
            data[core * BLOC : (core + 1) * BLOC].reshape(BLOC * P, D)
        )
        xn_pack = np.ascontiguousarray(
            xn.reshape(2 * BLOC, 128, D).transpose(1, 0, 2).reshape(128, -1)
        )
        av = 0.5 * ((xn ** 2).sum(axis=1) + ys ** 2).astype(np.float32)
        # winner-targeted rebase: scale a by lambda ~= C_min/A (per batch,
        # estimated from a task subsample) so the scan state stays small for
        # the low-C tasks that dominate the posterior -> ~5x less fp32
        # rounding noise where it matters
        for j in range(BLOC):
            b = core * BLOC + j
            es = Wsub @ data[b].T - targets[b][None, :]
            Cs = 0.5 * (es ** 2).sum(axis=1)
            Ab = av[j * P : (j + 1) * P].sum()
            lam = np.float32(Cs.min() / (2.0 * Ab))
            av[j * P : (j + 1) * P] *= 2.0 * lam
        amask = np.broadcast_to(av[None, :], (128, BLOC * P)).copy()
        in_maps.append(
            {"whl": whl, "wh2": wh2, "xhh": xhh, "xl2": xl2, "tp32": tp_pack,
             "x_nat": xn_pack, "amask": amask, "ident": ident}
        )
    return in_maps


def run(data, targets, task_pool, trace=False):
    nc = _get_nc()
    in_maps = _make_in_maps(data, targets, task_pool)
    res = bass_utils.run_bass_kernel_spmd(
        nc, in_maps, core_ids=list(range(NCORES)), trace=trace
    )
    out = np.empty((B, P), np.float32)
    for core in range(NCORES):
        out[core * BLOC : (core + 1) * BLOC] = res.results[core]["out"]
    return out, res


def kernel(data, targets, task_pool):
    out, _ = run(data, targets, task_pool)
    return out


# revision 48
# speedup vs baseline: 1.3527x; 1.0046x over previous
"""DiscreteMMSE Trainium2 kernel (v12).

Math (per batch b, sharded 4 batches/core over 8 cores):
  W = task_pool[:,:,0]                        # (T, D)
  err  = (W@x - y)/sqrt(2)   (PE fp16 hi/lo: [Wh;Wl]@[xh;xh] K=128 +
                              [Wh;1;1]@[xl;-yh;-yl] K=66; residual ~2^-22;
                              W,y pre-scaled by 1/sqrt(2) on host)
  sq   = err^2               (ACT Square, PSUM in -> SBUF out)
  nC   = cumsum_p (a - sq)   (DVE tensor_tensor_scan per chunk section:
                              state=(a+state)-sq; a(j)=(|x_j|^2+y_j^2)/2 is a
                              per-point rebase that cancels in the softmax but
                              keeps the fp32 scan state ~5x smaller = ~5x less
                              rounding noise than the reference's own cumsum)
  -m(p)= max_t nC(t,p)       (Pool: gpsimd.tensor_reduce(axis=C) per quad
                              into rows of a per-group stack tile, then
                              partition_all_reduce(max) across quads, then
                              3 tiny DVE folds over the 4 chunk sections and
                              gpsimd.partition_broadcast -> nmB (128,P))
  cs   = nC - (-m) = m - C   (DVE TT subtract, or PE fp32 ident + fp16
                              rank-1 (+m) into PSUM, by knob)
  e    = exp(+cs) fp16       (ACT, shifted: e[:,s,1:256]=exp(cs[:,s,0:255]),
                              col 0 preset to 1 == uniform posterior at p=0)
  ws   = sum_t e(t,p)*[w_t|1]  (PE fp16: TP (128,65) stationary, e moving,
                              (65,256) PSUM accum over 32 chunks)
  out(p) = (x_p . ws[0:64,p]) / ws[64,p]  (TT prod + ones-matmul + recip)
The shift by m cancels exactly in the num/den ratio; cs <= 0 so exp never
overflows and den >= 1.

Sharding: data-parallel over batch: 32 batches -> 8 cores x 4. No collectives.
"""

import os
import sys

sys.path.insert(0, "/opt/trn_rl_repo")
sys.path.insert(0, "/opt/trn_rl_repo/concourse")

import numpy as np

import concourse.bass as bass
import concourse.tile as tile
from concourse import bacc, bass_isa, bass_utils, mybir

F32 = mybir.dt.float32
F16 = mybir.dt.float16
AF = mybir.ActivationFunctionType
ALU = mybir.AluOpType
AX = mybir.AxisListType

B, P, D, T = 32, 256, 64, 4096
NCORES = 8
BLOC = B // NCORES          # 4 batches per core = 4 groups
NCH = T // 128              # 32 task chunks
NQ = NCH // 4               # 8 quads (4 chunks each) per group
QW = 4 * P                  # quad tile width (1024)
DA = D + 1

# tuning knobs
SUB_PE = int(os.environ.get("KSBP", "7"))    # quads (of 32) subtracted on PE
SUB_POOL = int(os.environ.get("KSBL", "0"))   # of the rest, quads on Pool
B_CS = int(os.environ.get("KBCS", "2"))
B_E = int(os.environ.get("KBE", "3"))
B_SQ = int(os.environ.get("KBSQ", "4"))
B_CP = int(os.environ.get("KBCP", "12"))
EARLY = int(os.environ.get("KEARLY", "3"))
PRIO_MIN = int(os.environ.get("KPRIO", "40"))
DBG = int(os.environ.get("KDBG", "0"))        # dump group-0 intermediates


def build_program(tc):
    nc = tc.nc

    whl_dram = nc.dram_tensor("whl", (128, T), F16, kind="ExternalInput").ap()
    wh2_dram = nc.dram_tensor("wh2", (66, T), F16, kind="ExternalInput").ap()
    xhh_dram = nc.dram_tensor("xhh", (128, BLOC * P), F16, kind="ExternalInput").ap()
    xl2_dram = nc.dram_tensor("xl2", (66, BLOC * P), F16, kind="ExternalInput").ap()
    xn_dram = nc.dram_tensor("x_nat", (128, 2 * BLOC * D), F32, kind="ExternalInput").ap()
    am_dram = nc.dram_tensor("amask", (128, BLOC * P), F32, kind="ExternalInput").ap()
    tp_dram = nc.dram_tensor("tp32", (128, NCH * DA), F32, kind="ExternalInput").ap()
    id_dram = nc.dram_tensor("ident", (128, 128), F32, kind="ExternalInput").ap()
    out_dram = nc.dram_tensor("out", (BLOC, P), F32, kind="ExternalOutput").ap()
    if DBG:
        dbg_nc = nc.dram_tensor("dbg_nc", (128, QW), F32, kind="ExternalOutput").ap()
        dbg_stk = nc.dram_tensor("dbg_stk", (NQ, QW), F32, kind="ExternalOutput").ap()
        dbg_nmb = nc.dram_tensor("dbg_nmb", (128, P), F32, kind="ExternalOutput").ap()
        dbg_e = nc.dram_tensor("dbg_e", (NQ, 128, QW), F32, kind="ExternalOutput").ap()
        dbg_ws = nc.dram_tensor("dbg_ws", (128, 2 * DA), F32, kind="ExternalOutput").ap()

    from contextlib import ExitStack

    with ExitStack() as ctx:
        consts = ctx.enter_context(tc.tile_pool(name="consts", bufs=1))
        sqp = ctx.enter_context(tc.tile_pool(name="sqp", bufs=B_SQ))
        cp = ctx.enter_context(tc.tile_pool(name="cp", bufs=B_CP))
        stp = ctx.enter_context(tc.tile_pool(name="stp", bufs=2))
        csp = ctx.enter_context(tc.tile_pool(name="csp", bufs=B_CS))
        ep = ctx.enter_context(tc.tile_pool(name="ep", bufs=B_E))
        sm = ctx.enter_context(tc.tile_pool(name="sm", bufs=2))
        pq = ctx.enter_context(tc.tile_pool(name="pq", bufs=3, space="PSUM"))
        wsp = ctx.enter_context(tc.tile_pool(name="wsp", bufs=2, space="PSUM"))

        # ---- constants / inputs ----
        WHL = consts.tile([128, T], F16, tag="whl", name="whl")
        WH2 = consts.tile([66, T], F16, tag="wh2", name="wh2")
        XHH = consts.tile([128, BLOC * P], F16, tag="xhh", name="xhh")
        XL2 = consts.tile([66, BLOC * P], F16, tag="xl2", name="xl2")
        TP_sb = consts.tile([128, NCH, DA], F32, tag="tpsb", name="tpsb")
        XN = consts.tile([128, 2 * BLOC, D], F32, tag="xn", name="xn")
        AMg = [consts.tile([128, P], F32, tag=f"am{g}", name=f"am{g}")
               for g in range(BLOC)]
        ID = consts.tile([128, 128], F32, tag="ident", name="ident")
        ONR = consts.tile([1, 128], F32, tag="onr", name="onr")


        # ordered by first use: quad (g0,q0) needs XHH/XL2/WHL0/WH20, the
        # first scan needs AM; bulk/late tensors go via the ACT DGE queue so
        # the SP sequencer is free for the per-quad stack-bounce DMAs
        # HWDGE and the DMA engines serialize in acquire order, so order the
        # DMAs by first use: mm1(g0,q0) needs XHH+WHL cols 0:1024, mm2 needs
        # XL2+WH2, the first scan needs AM[g0]
        nc.sync.dma_start(XHH[:], xhh_dram)
        nc.sync.dma_start(WHL[:, 0:1024], whl_dram[:, 0:1024])
        nc.sync.dma_start(XL2[:], xl2_dram)
        nc.sync.dma_start(WH2[:, 0:1024], wh2_dram[:, 0:1024])
        nc.sync.dma_start(AMg[0][:], am_dram[:, 0:P])
        nc.sync.dma_start(WHL[:, 1024:], whl_dram[:, 1024:])
        nc.sync.dma_start(WH2[:, 1024:], wh2_dram[:, 1024:])
        for g in range(1, BLOC):
            nc.sync.dma_start(AMg[g][:], am_dram[:, g * P : (g + 1) * P])
        nc.gpsimd.memset(ONR[:], 1.0)
        late_dma = [False]

        def emit_late_dmas():
            # ID/XN/TP are first needed at p15(0)/p2(0)/p3(0); emitting them
            # here (mid group 0) keeps the ACT sequencer free for the first
            # squares at startup
            if late_dma[0]:
                return
            late_dma[0] = True
            with tc.tile_wait_until(0.012):
                nc.scalar.dma_start(ID[:], id_dram)
                nc.scalar.dma_start(
                    XN[:], xn_dram.rearrange("q (j d) -> q j d", d=D))
                nc.scalar.dma_start(
                    TP_sb[:], tp_dram.rearrange("q (c d) -> q c d", d=DA))

        # e ring: fp16, col 0 of each 256-section preset to 1.0 (p=0 uniform)
        e_ring = []
        for i in range(B_E):
            t = ep.tile([128, QW], F32, tag=f"e{i}", name=f"e{i}")
            for k in range(4):
                nc.gpsimd.memset(t[:, k * P : k * P + 1], 1.0)
            e_ring.append(t)

        c_tiles = {}
        stack_t = {}
        nm_t = {}
        ws_tiles = {}
        cnt_sub = [0]

        def p1_quad(g, q):
            # phase 1: err -> sq -> scan(nC) -> Pool partition-max into stack
            errq = pq.tile([128, QW], F32, tag="eq", name="err")
            for k in range(4):
                c = 4 * q + k
                sl = slice(k * P, (k + 1) * P)
                nc.tensor.matmul(
                    errq[:, sl],
                    lhsT=WHL[:, c * 128 : (c + 1) * 128],
                    rhs=XHH[:, g * P : (g + 1) * P],
                    start=True, stop=False, skip_group_check=True,
                )
                nc.tensor.matmul(
                    errq[:, sl],
                    lhsT=WH2[:, c * 128 : (c + 1) * 128],
                    rhs=XL2[:, g * P : (g + 1) * P],
                    start=False, stop=True, skip_group_check=True,
                )
            sq = sqp.tile([128, QW], F32, tag="sq", name="sq")
            nc.scalar.activation(sq[:], errq[:], AF.Square, bias=0.0, scale=1.0)

            nC = cp.tile([128, QW], F32, tag="c", name="c")
            c_tiles[(g, q)] = nC
            amg = AMg[g][:]
            from contextlib import nullcontext
            pctx = (tc.high_priority(PRIO_MIN) if g == BLOC - 1
                    else nullcontext())
            with pctx:
                for s in range(4):
                    nc.vector.tensor_tensor_scan(
                        nC[:, s * P : (s + 1) * P], amg,
                        sq[:, s * P : (s + 1) * P],
                        0.0, op0=ALU.add, op1=ALU.subtract,
                    )
            if DBG and g == 0 and q == 0:
                nc.sync.dma_start(dbg_nc, nC[:])
            # per-quad partition max (over the 128 tasks of each chunk row);
            # gpsimd C-reduce must write partition 0, so bounce via DMA into
            # the per-group stack row
            stk = stack_t[g]
            ctmp = stp.tile([1, QW], F32, tag="ctmp", name="ctmp", bufs=3)
            with tc.high_priority(PRIO_MIN):
                nc.gpsimd.tensor_reduce(ctmp[:], nC[:], axis=AX.C, op=ALU.max)
                nc.sync.dma_start(stk[4 * q : 4 * (q + 1), :], ctmp[:])

        def p15(g, prio=None):
            # cross-quad max, fold 4 chunk-sections, broadcast -m
            ctx15 = tc.high_priority(PRIO_MIN if prio is None else prio)
            ctx15.__enter__()
            stk = stack_t[g]
            stk2 = stp.tile([4 * NQ, P], F32, tag="stk2", name=f"stk2_{g}")
            nc.gpsimd.partition_all_reduce(
                stk2[:], stk[:], channels=4 * NQ, reduce_op=bass_isa.ReduceOp.max
            )
            nmB = sm.tile([128, P], F32, tag=f"nmB{g % 2}", name=f"nmB{g}")
            nc.gpsimd.partition_broadcast(nmB[:], stk2[0:1, :], channels=128)
            mh = None
            if SUB_PE > 0:
                mh = sm.tile([1, P], F32, tag=f"mh{g % 2}", name=f"mh{g}")
                nc.vector.tensor_scalar_mul(mh[:], stk2[0:1, :], -1.0)
            nm_t[g] = (nmB, mh)
            ctx15.__exit__(None, None, None)
            if DBG and g == 0:
                nc.sync.dma_start(dbg_stk, stk2[:])
                nc.sync.dma_start(dbg_nmb, nmB[:])
            wsb = wsp.tile([128, 2 * DA], F32, tag="wsj", name=f"ws{g}")
            ws_tiles[g] = wsb

        def p2_quad(g, q):
            # phase 2: cs = nC - (-m) = m - C; e = exp(cs) shifted; ws accum
            nC = c_tiles.pop((g, q))
            nmB, mh = nm_t[g]
            j = cnt_sub[0]
            cnt_sub[0] += 1
            on_pe = (j * SUB_PE) // 32 != ((j + 1) * SUB_PE) // 32
            on_pool = (not on_pe and
                       (j * SUB_POOL) // 32 != ((j + 1) * SUB_POOL) // 32)
            if on_pe:
                cs = pq.tile([128, QW], F32, tag="eq", name="cs_ps")
                Cv = nC[:].rearrange("p (s x) -> p s x", x=P)
                for h in range(2):
                    sl = slice(h * 512, (h + 1) * 512)
                    nc.tensor.matmul(
                        cs[:, sl], lhsT=ID[:], rhs=Cv[:, 2 * h : 2 * h + 2, :],
                        start=True, stop=False, skip_group_check=True,
                    )
                    for hh in range(2):
                        nc.tensor.matmul(
                            cs[:, (2 * h + hh) * P : (2 * h + hh + 1) * P],
                            lhsT=ONR[:], rhs=mh[:],
                            start=False, stop=(hh == 1), skip_group_check=True,
                        )
            else:
                cs = csp.tile([128, QW], F32, tag="cs", name="cs")
                csv = cs[:].rearrange("p (s x) -> p s x", x=P)
                Cv = nC[:].rearrange("p (s x) -> p s x", x=P)
                nmv = (nmB[:].rearrange("p (a x) -> p a x", a=1)
                       .broadcast_to([128, 4, P]))
                if on_pool:
                    nc.gpsimd.scalar_tensor_tensor(
                        csv, Cv, 1.0, nmv, op0=ALU.mult, op1=ALU.subtract
                    )
                else:
                    nc.vector.tensor_tensor(csv, Cv, nmv, op=ALU.subtract)
            e = e_ring[(g * NQ + q) % B_E]
            ev = e[:].rearrange("p (s x) -> p s x", x=P)[:, :, 1:P]
            csv2 = cs[:].rearrange("p (s x) -> p s x", x=P)[:, :, 0 : P - 1]
            nc.scalar.activation(ev, csv2, AF.Exp, bias=0.0, scale=1.0)
            if DBG and g == 0:
                nc.sync.dma_start(dbg_e[q], e[:])
            wsb = ws_tiles[g]
            for k in range(4):
                c = 4 * q + k
                for j in range(2):
                    nc.tensor.matmul(
                        wsb[:, j * DA : (j + 1) * DA],
                        lhsT=e[:, k * P + j * 128 : k * P + (j + 1) * 128],
                        rhs=TP_sb[:, c, :],
                        start=(c == 0 and j == 0), stop=(c == NCH - 1),
                        skip_group_check=True,
                    )

        def p3(g):
            # out(q-lane, j) = (x . wsj[q, 0:64]) / wsj[q, 64]
            wsb = ws_tiles.pop(g)
            if DBG and g == 0:
                ws_dbg = sm.tile([128, 2 * DA], F32, tag="wsdbg", name="wsdbg")
                nc.vector.tensor_copy(ws_dbg[:], wsb[:])
                nc.sync.dma_start(dbg_ws, ws_dbg[:])
            nrg = sm.tile([128, 2], F32, tag="nrg", name="nrg")
            dcol = sm.tile([128, 2], F32, tag="dcol", name="dcol")
            for j in range(2):
                prod = sm.tile([128, D], F32, tag="prod", name="prod")
                nc.vector.tensor_tensor(
                    prod[:], XN[:, g * 2 + j, :], wsb[:, j * DA : j * DA + D],
                    op=ALU.mult,
                )
                nc.vector.tensor_reduce(
                    nrg[:, j : j + 1], prod[:], axis=AX.X, op=ALU.add
                )
                nc.vector.tensor_copy(
                    dcol[:, j : j + 1], wsb[:, j * DA + D : j * DA + D + 1]
                )
            rden = sm.tile([128, 2], F32, tag="rden", name="rden")
            nc.vector.reciprocal(rden[:], dcol[:])
            o = sm.tile([128, 2], F32, tag="o", name="o")
            nc.vector.tensor_tensor(o[:], nrg[:], rden[:], op=ALU.mult)
            nc.sync.dma_start(
                out_dram[g : g + 1, :].rearrange("b (h q) -> q (b h)", q=128),
                o[:],
            )

        # software-pipelined emission: interleave group g's phase-1 with
        # group g-1's phase-2; min-finalize (p15) hides behind early quads
        for g in range(BLOC):
            stack_t[g] = stp.tile([4 * NQ, P], F32, tag="stk", name=f"stk{g}")
            for q in range(NQ):
                p1_quad(g, q)
                if g == 0 and q == 2:
                    emit_late_dmas()
                if q == 0 and g > 0:
                    p15(g - 1)
                if g > 0 and q >= EARLY:
                    p2_quad(g - 1, q - EARLY)
            if g > 0:
                for q in range(NQ - EARLY, NQ):
                    p2_quad(g - 1, q)
                p3(g - 1)
        p15(BLOC - 1)
        for q in range(NQ):
            p2_quad(BLOC - 1, q)
        p3(BLOC - 1)


_CACHE = {}


def _get_nc():
    if "nc" not in _CACHE:
        nc = bacc.Bacc(
            "TRN2",
            target_bir_lowering=False,
            debug=False,
            enable_asserts=False,
            num_devices=NCORES,
        )
        with tile.TileContext(nc) as tc:
            build_program(tc)
        nc.compile()
        _CACHE["nc"] = nc
    return _CACHE["nc"]


def _split_pair(a):
    hi = a.astype(np.float16)
    lo = (a - hi.astype(np.float32)).astype(np.float16)
    return hi, lo


def _make_in_maps(data, targets, task_pool):
    data = np.ascontiguousarray(data, dtype=np.float32)
    targets = np.ascontiguousarray(targets, dtype=np.float32)
    task_pool = np.ascontiguousarray(task_pool, dtype=np.float32)
    isq2 = np.float32(1.0 / np.sqrt(2.0))
    W = task_pool[:, :, 0]  # (T, D)
    Ws = W.T * isq2  # (D, T), pre-scaled so sq = err^2 directly
    wh, wl = _split_pair(Ws)
    whl = np.concatenate([wh, wl], axis=0)  # (128, T) fp16
    wh2 = np.concatenate(
        [wh, np.ones((2, T), np.float16)], axis=0
    )  # (66, T): [Wh; 1; 1]
    tp32 = np.concatenate(
        [W, np.ones((T, 1), np.float32)], axis=1
    )  # (T, 65) fp32
    tp_pack = np.ascontiguousarray(
        tp32.reshape(NCH, 128, DA).transpose(1, 0, 2).reshape(128, -1)
    )
    ident = np.eye(128, dtype=np.float32)
    Wsub = W[::16]  # (256, D) deterministic subsample for lambda estimate
    in_maps = []
    for core in range(NCORES):
        xs = np.empty((D, BLOC * P), np.float32)
        ys = np.empty((BLOC * P,), np.float32)
        for j in range(BLOC):
            b = core * BLOC + j
            xs[:, j * P : (j + 1) * P] = data[b].T
            ys[j * P : (j + 1) * P] = targets[b]
        xh, xl = _split_pair(xs)
        xhh = np.concatenate([xh, xh], axis=0)  # (128, 1024)
        nys = -ys * isq2
        nyh, nyl = _split_pair(nys)
        xl2 = np.concatenate(
            [xl, nyh[None, :], nyl[None, :]], axis=0
        )  # (66, 1024): [xl; -yh; -yl]
        xn = np.ascontiguousarray(
            data[core * BLOC : (core + 1) * BLOC].reshape(BLOC * P, D)
        )
        xn_pack = np.ascontiguousarray(
            xn.reshape(2 * BLOC, 128, D).transpose(1, 0, 2).reshape(128, -1)
        )
        av = 0.5 * ((xn ** 2).sum(axis=1) + ys ** 2).astype(np.float32)
        # winner-targeted rebase: scale a by lambda ~= C_min/A (per batch,
        # estimated from a task subsample) so the scan state stays small for
        # the low-C tasks that dominate the posterior -> ~5x less fp32
        # rounding noise where it matters
        for j in range(BLOC):
            b = core * BLOC + j
            es = Wsub @ data[b].T - targets[b][None, :]
            Cs = 0.5 * (es ** 2).sum(axis=1)
            Ab = av[j * P : (j + 1) * P].sum()
            lam = np.float32(Cs.min() / (2.0 * Ab))
            av[j * P : (j + 1) * P] *= 2.0 * lam
        amask = np.broadcast_to(av[None, :], (128, BLOC * P)).copy()
        in_maps.append(
            {"whl": whl, "wh2": wh2, "xhh": xhh, "xl2": xl2, "tp32": tp_pack,
             "x_nat": xn_pack, "amask": amask, "ident": ident}
        )
    return in_maps


def run(data, targets, task_pool, trace=False):
    nc = _get_nc()
    in_maps = _make_in_maps(data, targets, task_pool)
    res = bass_utils.run_bass_kernel_spmd(
        nc, in_maps, core_ids=list(range(NCORES)), trace=trace
    )
    out = np.empty((B, P), np.float32)
    for core in range(NCORES):
        out[core * BLOC : (core + 1) * BLOC] = res.results[core]["out"]
    return out, res


def kernel(data, targets, task_pool):
    out, _ = run(data, targets, task_pool)
    return out


# revision 67
# speedup vs baseline: 1.3964x; 1.0323x over previous
"""DiscreteMMSE Trainium2 kernel (v14).

Math (per batch b, data-parallel 4 batches/core over 8 cores):
  W = task_pool[:,:,0]                            # (T, D)
  err  = (W@x - y)/sqrt(2)      PE fp16 hi/lo pair: [Wh;Wl]@[xh;xh] (K=128)
                                + [Wh;1;1]@[xl;-yh;-yl] (K=66); residual
                                ~2^-22; W,y pre-scaled by 1/sqrt(2) on host.
  sq   = err^2                  ACT Square (PSUM in, SBUF out).
  nC   = cumsum_p (a - sq)      DVE tensor_tensor_scan per 256-pt chunk
                                section: state=(a+state)-sq. a(j) =
                                lambda_b*(|x_j|^2+y_j^2)/2 is a per-point
                                rebase (cancels in the softmax) with lambda_b
                                ~= C_min/A estimated host-side from a task
                                subsample, so the scan state stays ~small FOR
                                THE LOW-C TASKS THAT DOMINATE THE POSTERIOR
                                -> ~5x less fp32 rounding noise than the
                                reference's own cumsum at the tasks that
                                matter (knife-edge softmax points).
  -m(p)= max_t nC(t,p)          Pool gpsimd.tensor_reduce(axis=C) per quad
                                -> (1,1024), DMA-scattered into 4 partition
                                rows of a (32,256) per-group stack, one
                                partition_all_reduce(max) collapses quads AND
                                chunk sections, partition_broadcast -> nmB.
  cs   = nC - nmB = m - C       DVE TT subtract (bcast view), or PE fp32
                                ident + rank-1 (+m) into PSUM (KSBP quads;
                                all-fp32: mixed-dtype PSUM groups are broken
                                on HW).
  e    = exp(cs) fp32           ACT, shifted view e[:,s,1:256] =
                                exp(cs[:,s,0:255]); col 0 preset to 1
                                (uniform posterior at p=0). cs <= 0 so exp
                                never overflows and den >= 1; e/W/x stay fp32
                                through the output stage (fp16 there fails
                                the rel-err gate at near-zero outputs).
  ws   = sum_t e(t,p)*[w_t|1]   PE fp32, e chunks stationary, TP (128,65)
                                moving, (128,130) PSUM accum over 32 chunks.
  out(p) = (x_p . ws[0:64,p]) / ws[64,p]   DVE prod/reduce/recip per group.

DMAs are ordered by first use on one queue (HWDGE+DMA engines serialize in
acquire order); TP/XN are host-prepacked so their transfers are contiguous;
late consts ride the ACT DGE queue behind a wait-until.

Sharding: data-parallel over batch: 32 batches -> 8 cores x 4. No collectives.
"""

import os
import sys

sys.path.insert(0, "/opt/trn_rl_repo")
sys.path.insert(0, "/opt/trn_rl_repo/concourse")

import numpy as np

import concourse.bass as bass
import concourse.tile as tile
from concourse import bacc, bass_isa, bass_utils, mybir

F32 = mybir.dt.float32
F16 = mybir.dt.float16
AF = mybir.ActivationFunctionType
ALU = mybir.AluOpType
AX = mybir.AxisListType

B, P, D, T = 32, 256, 64, 4096
NCORES = 8
BLOC = B // NCORES          # 4 batches per core = 4 groups
NCH = T // 128              # 32 task chunks
NQ = NCH // 4               # 8 quads (4 chunks each) per group
QW = 4 * P                  # quad tile width (1024)
DA = D + 1

# tuning knobs
SUB_PE = int(os.environ.get("KSBP", "3"))    # quads (of 32) subtracted on PE
SUB_POOL = int(os.environ.get("KSBL", "0"))   # of the rest, quads on Pool
B_CS = int(os.environ.get("KBCS", "3"))
B_E = int(os.environ.get("KBE", "3"))
B_SQ = int(os.environ.get("KBSQ", "4"))
B_CP = int(os.environ.get("KBCP", "12"))
EARLY = int(os.environ.get("KEARLY", "3"))
PRIO_MIN = int(os.environ.get("KPRIO", "40"))
DBG = int(os.environ.get("KDBG", "0"))        # dump group-0 intermediates


def build_program(tc):
    nc = tc.nc

    whl_dram = nc.dram_tensor("whl", (128, T), F16, kind="ExternalInput").ap()
    wh2_dram = nc.dram_tensor("wh2", (66, T), F16, kind="ExternalInput").ap()
    xhh_dram = nc.dram_tensor("xhh", (128, BLOC * P), F16, kind="ExternalInput").ap()
    xl2_dram = nc.dram_tensor("xl2", (66, BLOC * P), F16, kind="ExternalInput").ap()
    xn_dram = nc.dram_tensor("x_nat", (128, 2 * BLOC * D), F32, kind="ExternalInput").ap()
    am_dram = nc.dram_tensor("amask", (128, BLOC * P), F32, kind="ExternalInput").ap()
    tp_dram = nc.dram_tensor("tp32", (128, NCH * DA), F32, kind="ExternalInput").ap()
    id_dram = nc.dram_tensor("ident", (128, 128), F32, kind="ExternalInput").ap()
    out_dram = nc.dram_tensor("out", (BLOC, P), F32, kind="ExternalOutput").ap()
    if DBG:
        dbg_nc = nc.dram_tensor("dbg_nc", (128, QW), F32, kind="ExternalOutput").ap()
        dbg_stk = nc.dram_tensor("dbg_stk", (NQ, QW), F32, kind="ExternalOutput").ap()
        dbg_nmb = nc.dram_tensor("dbg_nmb", (128, P), F32, kind="ExternalOutput").ap()
        dbg_e = nc.dram_tensor("dbg_e", (NQ, 128, QW), F32, kind="ExternalOutput").ap()
        dbg_ws = nc.dram_tensor("dbg_ws", (128, 2 * DA), F32, kind="ExternalOutput").ap()

    from contextlib import ExitStack

    with ExitStack() as ctx:
        consts = ctx.enter_context(tc.tile_pool(name="consts", bufs=1))
        sqp = ctx.enter_context(tc.tile_pool(name="sqp", bufs=B_SQ))
        cp = ctx.enter_context(tc.tile_pool(name="cp", bufs=B_CP))
        stp = ctx.enter_context(tc.tile_pool(name="stp", bufs=2))
        csp = ctx.enter_context(tc.tile_pool(name="csp", bufs=B_CS))
        ep = ctx.enter_context(tc.tile_pool(name="ep", bufs=B_E))
        sm = ctx.enter_context(tc.tile_pool(name="sm", bufs=2))
        pq = ctx.enter_context(tc.tile_pool(name="pq", bufs=3, space="PSUM"))
        wsp = ctx.enter_context(tc.tile_pool(name="wsp", bufs=2, space="PSUM"))

        # ---- constants / inputs ----
        WHL = consts.tile([128, T], F16, tag="whl", name="whl")
        WH2 = consts.tile([66, T], F16, tag="wh2", name="wh2")
        XHH = consts.tile([128, BLOC * P], F16, tag="xhh", name="xhh")
        XL2 = consts.tile([66, BLOC * P], F16, tag="xl2", name="xl2")
        TP_sb = consts.tile([128, NCH, DA], F32, tag="tpsb", name="tpsb")
        XN = consts.tile([128, 2 * BLOC, D], F32, tag="xn", name="xn")
        AMg = [consts.tile([128, P], F32, tag=f"am{g}", name=f"am{g}")
               for g in range(BLOC)]
        ID = consts.tile([128, 128], F32, tag="ident", name="ident")
        ONR = consts.tile([1, 128], F32, tag="onr", name="onr")


        # ordered by first use: quad (g0,q0) needs XHH/XL2/WHL0/WH20, the
        # first scan needs AM; bulk/late tensors go via the ACT DGE queue so
        # the SP sequencer is free for the per-quad stack-bounce DMAs
        # HWDGE and the DMA engines serialize in acquire order, so order the
        # DMAs by first use: mm1(g0,q0) needs XHH+WHL cols 0:1024, mm2 needs
        # XL2+WH2, the first scan needs AM[g0]
        nc.sync.dma_start(XHH[:], xhh_dram)
        nc.sync.dma_start(WHL[:, 0:1024], whl_dram[:, 0:1024])
        nc.sync.dma_start(XL2[:], xl2_dram)
        nc.sync.dma_start(WH2[:, 0:1024], wh2_dram[:, 0:1024])
        nc.sync.dma_start(AMg[0][:], am_dram[:, 0:P])
        nc.sync.dma_start(WHL[:, 1024:], whl_dram[:, 1024:])
        nc.sync.dma_start(WH2[:, 1024:], wh2_dram[:, 1024:])
        for g in range(1, BLOC):
            nc.sync.dma_start(AMg[g][:], am_dram[:, g * P : (g + 1) * P])
        nc.gpsimd.memset(ONR[:], 1.0)
        late_dma = [False]

        def emit_late_dmas():
            # ID/XN/TP are first needed at p15(0)/p2(0)/p3(0); emitting them
            # here (mid group 0) keeps the ACT sequencer free for the first
            # squares at startup
            if late_dma[0]:
                return
            late_dma[0] = True
            with tc.tile_wait_until(0.012):
                nc.scalar.dma_start(ID[:], id_dram)
                nc.scalar.dma_start(
                    XN[:], xn_dram.rearrange("q (j d) -> q j d", d=D))
                nc.scalar.dma_start(
                    TP_sb[:], tp_dram.rearrange("q (c d) -> q c d", d=DA))

        # e ring: fp16, col 0 of each 256-section preset to 1.0 (p=0 uniform)
        e_ring = []
        for i in range(B_E):
            t = ep.tile([128, QW], F32, tag=f"e{i}", name=f"e{i}")
            for k in range(4):
                nc.gpsimd.memset(t[:, k * P : k * P + 1], 1.0)
            e_ring.append(t)

        c_tiles = {}
        stack_t = {}
        nm_t = {}
        ws_tiles = {}
        cnt_sub = [0]

        def p1_quad(g, q):
            # phase 1: err -> sq -> scan(nC) -> Pool partition-max into stack
            errq = pq.tile([128, QW], F32, tag="eq", name="err")
            for k in range(4):
                c = 4 * q + k
                sl = slice(k * P, (k + 1) * P)
                nc.tensor.matmul(
                    errq[:, sl],
                    lhsT=WHL[:, c * 128 : (c + 1) * 128],
                    rhs=XHH[:, g * P : (g + 1) * P],
                    start=True, stop=False, skip_group_check=True,
                )
                nc.tensor.matmul(
                    errq[:, sl],
                    lhsT=WH2[:, c * 128 : (c + 1) * 128],
                    rhs=XL2[:, g * P : (g + 1) * P],
                    start=False, stop=True, skip_group_check=True,
                )
            sq = sqp.tile([128, QW], F32, tag="sq", name="sq")
            nc.scalar.activation(sq[:], errq[:], AF.Square, bias=0.0, scale=1.0)

            nC = cp.tile([128, QW], F32, tag="c", name="c")
            c_tiles[(g, q)] = nC
            amg = AMg[g][:]
            from contextlib import nullcontext
            pctx = (tc.high_priority(PRIO_MIN) if g == BLOC - 1
                    else nullcontext())
            with pctx:
                for s in range(4):
                    nc.vector.tensor_tensor_scan(
                        nC[:, s * P : (s + 1) * P], amg,
                        sq[:, s * P : (s + 1) * P],
                        0.0, op0=ALU.add, op1=ALU.subtract,
                    )
            if DBG and g == 0 and q == 0:
                nc.sync.dma_start(dbg_nc, nC[:])
            # per-quad partition max (over the 128 tasks of each chunk row);
            # gpsimd C-reduce must write partition 0, so bounce via DMA into
            # the per-group stack row
            stk = stack_t[g]
            ctmp = stp.tile([1, QW], F32, tag="ctmp", name="ctmp", bufs=3)
            with tc.high_priority(PRIO_MIN):
                nc.gpsimd.tensor_reduce(ctmp[:], nC[:], axis=AX.C, op=ALU.max)
                nc.sync.dma_start(stk[4 * q : 4 * (q + 1), :], ctmp[:])

        def p15(g, prio=None):
            # cross-quad max, fold 4 chunk-sections, broadcast -m
            ctx15 = tc.high_priority(PRIO_MIN if prio is None else prio)
            ctx15.__enter__()
            stk = stack_t[g]
            stk2 = stp.tile([4 * NQ, P], F32, tag="stk2", name=f"stk2_{g}")
            nc.gpsimd.partition_all_reduce(
                stk2[:], stk[:], channels=4 * NQ, reduce_op=bass_isa.ReduceOp.max
            )
            nmB = sm.tile([128, P], F32, tag=f"nmB{g % 2}", name=f"nmB{g}")
            nc.gpsimd.partition_broadcast(nmB[:], stk2[0:1, :], channels=128)
            mh = None
            if SUB_PE > 0:
                mh = sm.tile([1, P], F32, tag=f"mh{g % 2}", name=f"mh{g}")
                nc.vector.tensor_scalar_mul(mh[:], stk2[0:1, :], -1.0)
            nm_t[g] = (nmB, mh)
            ctx15.__exit__(None, None, None)
            if DBG and g == 0:
                nc.sync.dma_start(dbg_stk, stk2[:])
                nc.sync.dma_start(dbg_nmb, nmB[:])
            wsb = wsp.tile([128, 2 * DA], F32, tag="wsj", name=f"ws{g}")
            ws_tiles[g] = wsb

        def p2_quad(g, q):
            # phase 2: cs = nC - (-m) = m - C; e = exp(cs) shifted; ws accum
            nC = c_tiles.pop((g, q))
            nmB, mh = nm_t[g]
            j = cnt_sub[0]
            cnt_sub[0] += 1
            on_pe = (j * SUB_PE) // 32 != ((j + 1) * SUB_PE) // 32
            on_pool = (not on_pe and
                       (j * SUB_POOL) // 32 != ((j + 1) * SUB_POOL) // 32)
            if on_pe:
                cs = pq.tile([128, QW], F32, tag="eq", name="cs_ps")
                Cv = nC[:].rearrange("p (s x) -> p s x", x=P)
                for h in range(2):
                    sl = slice(h * 512, (h + 1) * 512)
                    nc.tensor.matmul(
                        cs[:, sl], lhsT=ID[:], rhs=Cv[:, 2 * h : 2 * h + 2, :],
                        start=True, stop=False, skip_group_check=True,
                    )
                    for hh in range(2):
                        nc.tensor.matmul(
                            cs[:, (2 * h + hh) * P : (2 * h + hh + 1) * P],
                            lhsT=ONR[:], rhs=mh[:],
                            start=False, stop=(hh == 1), skip_group_check=True,
                        )
            else:
                cs = csp.tile([128, QW], F32, tag="cs", name="cs")
                csv = cs[:].rearrange("p (s x) -> p s x", x=P)
                Cv = nC[:].rearrange("p (s x) -> p s x", x=P)
                nmv = (nmB[:].rearrange("p (a x) -> p a x", a=1)
                       .broadcast_to([128, 4, P]))
                if on_pool:
                    nc.gpsimd.scalar_tensor_tensor(
                        csv, Cv, 1.0, nmv, op0=ALU.mult, op1=ALU.subtract
                    )
                else:
                    nc.vector.tensor_tensor(csv, Cv, nmv, op=ALU.subtract)
            e = e_ring[(g * NQ + q) % B_E]
            ev = e[:].rearrange("p (s x) -> p s x", x=P)[:, :, 1:P]
            csv2 = cs[:].rearrange("p (s x) -> p s x", x=P)[:, :, 0 : P - 1]
            nc.scalar.activation(ev, csv2, AF.Exp, bias=0.0, scale=1.0)
            if DBG and g == 0:
                nc.sync.dma_start(dbg_e[q], e[:])
            wsb = ws_tiles[g]
            for k in range(4):
                c = 4 * q + k
                for j in range(2):
                    nc.tensor.matmul(
                        wsb[:, j * DA : (j + 1) * DA],
                        lhsT=e[:, k * P + j * 128 : k * P + (j + 1) * 128],
                        rhs=TP_sb[:, c, :],
                        start=(c == 0 and j == 0), stop=(c == NCH - 1),
                        skip_group_check=True,
                    )

        def p3(g):
            # out(q-lane, j) = (x . wsj[q, 0:64]) / wsj[q, 64]
            wsb = ws_tiles.pop(g)
            if DBG and g == 0:
                ws_dbg = sm.tile([128, 2 * DA], F32, tag="wsdbg", name="wsdbg")
                nc.vector.tensor_copy(ws_dbg[:], wsb[:])
                nc.sync.dma_start(dbg_ws, ws_dbg[:])
            nrg = sm.tile([128, 2], F32, tag="nrg", name="nrg")
            dcol = sm.tile([128, 2], F32, tag="dcol", name="dcol")
            for j in range(2):
                prod = sm.tile([128, D], F32, tag="prod", name="prod")
                nc.vector.tensor_tensor(
                    prod[:], XN[:, g * 2 + j, :], wsb[:, j * DA : j * DA + D],
                    op=ALU.mult,
                )
                nc.vector.tensor_reduce(
                    nrg[:, j : j + 1], prod[:], axis=AX.X, op=ALU.add
                )
                nc.vector.tensor_copy(
                    dcol[:, j : j + 1], wsb[:, j * DA + D : j * DA + D + 1]
                )
            rden = sm.tile([128, 2], F32, tag="rden", name="rden")
            nc.vector.reciprocal(rden[:], dcol[:])
            o = sm.tile([128, 2], F32, tag="o", name="o")
            nc.vector.tensor_tensor(o[:], nrg[:], rden[:], op=ALU.mult)
            nc.sync.dma_start(
                out_dram[g : g + 1, :].rearrange("b (h q) -> q (b h)", q=128),
                o[:],
            )

        # software-pipelined emission: interleave group g's phase-1 with
        # group g-1's phase-2; min-finalize (p15) hides behind early quads
        for g in range(BLOC):
            stack_t[g] = stp.tile([4 * NQ, P], F32, tag="stk", name=f"stk{g}")
            for q in range(NQ):
                p1_quad(g, q)
                if g == 0 and q == 2:
                    emit_late_dmas()
                if q == 0 and g > 0:
                    p15(g - 1)
                if g > 0 and q >= EARLY:
                    p2_quad(g - 1, q - EARLY)
            if g > 0:
                for q in range(NQ - EARLY, NQ):
                    p2_quad(g - 1, q)
                p3(g - 1)
        p15(BLOC - 1)
        for q in range(NQ):
            p2_quad(BLOC - 1, q)
        p3(BLOC - 1)


_CACHE = {}


def _get_nc():
    if "nc" not in _CACHE:
        nc = bacc.Bacc(
            "TRN2",
            target_bir_lowering=False,
            debug=False,
            enable_asserts=False,
            num_devices=NCORES,
        )
        with tile.TileContext(nc) as tc:
            build_program(tc)
        nc.compile()
        _CACHE["nc"] = nc
    return _CACHE["nc"]


def _split_pair(a):
    hi = a.astype(np.float16)
    lo = (a - hi.astype(np.float32)).astype(np.float16)
    return hi, lo


def _make_in_maps(data, targets, task_pool):
    data = np.ascontiguousarray(data, dtype=np.float32)
    targets = np.ascontiguousarray(targets, dtype=np.float32)
    task_pool = np.ascontiguousarray(task_pool, dtype=np.float32)
    isq2 = np.float32(1.0 / np.sqrt(2.0))
    W = task_pool[:, :, 0]  # (T, D)
    Ws = W.T * isq2  # (D, T), pre-scaled so sq = err^2 directly
    wh, wl = _split_pair(Ws)
    whl = np.concatenate([wh, wl], axis=0)  # (128, T) fp16
    wh2 = np.concatenate(
        [wh, np.ones((2, T), np.float16)], axis=0
    )  # (66, T): [Wh; 1; 1]
    tp32 = np.concatenate(
        [W, np.ones((T, 1), np.float32)], axis=1
    )  # (T, 65) fp32
    tp_pack = np.ascontiguousarray(
        tp32.reshape(NCH, 128, DA).transpose(1, 0, 2).reshape(128, -1)
    )
    ident = np.eye(128, dtype=np.float32)
    Wsub = W[::16]  # (256, D) deterministic subsample for lambda estimate
    in_maps = []
    for core in range(NCORES):
        xs = np.empty((D, BLOC * P), np.float32)
        ys = np.empty((BLOC * P,), np.float32)
        for j in range(BLOC):
            b = core * BLOC + j
            xs[:, j * P : (j + 1) * P] = data[b].T
            ys[j * P : (j + 1) * P] = targets[b]
        xh, xl = _split_pair(xs)
        xhh = np.concatenate([xh, xh], axis=0)  # (128, 1024)
        nys = -ys * isq2
        nyh, nyl = _split_pair(nys)
        xl2 = np.concatenate(
            [xl, nyh[None, :], nyl[None, :]], axis=0
        )  # (66, 1024): [xl; -yh; -yl]
        xn = np.ascontiguousarray(
            data[core * BLOC : (core + 1) * BLOC].reshape(BLOC * P, D)
        )
        xn_pack = np.ascontiguousarray(
            xn.reshape(2 * BLOC, 128, D).transpose(1, 0, 2).reshape(128, -1)
        )
        av = 0.5 * ((xn ** 2).sum(axis=1) + ys ** 2).astype(np.float32)
        # winner-targeted rebase: scale a by lambda ~= C_min/A (per batch,
        # estimated from a task subsample) so the scan state stays small for
        # the low-C tasks that dominate the posterior -> ~5x less fp32
        # rounding noise where it matters
        for j in range(BLOC):
            b = core * BLOC + j
            es = Wsub @ data[b].T - targets[b][None, :]
            Cs = 0.5 * (es ** 2).sum(axis=1)
            Ab = av[j * P : (j + 1) * P].sum()
            lam = np.float32(Cs.min() / (2.0 * Ab))
            av[j * P : (j + 1) * P] *= 2.0 * lam
        amask = np.broadcast_to(av[None, :], (128, BLOC * P)).copy()
        in_maps.append(
            {"whl": whl, "wh2": wh2, "xhh": xhh, "xl2": xl2, "tp32": tp_pack,
             "x_nat": xn_pack, "amask": amask, "ident": ident}
        )
    return in_maps


def run(data, targets, task_pool, trace=False):
    nc = _get_nc()
    in_maps = _make_in_maps(data, targets, task_pool)
    res = bass_utils.run_bass_kernel_spmd(
        nc, in_maps, core_ids=list(range(NCORES)), trace=trace
    )
    out = np.empty((B, P), np.float32)
    for core in range(NCORES):
        out[core * BLOC : (core + 1) * BLOC] = res.results[core]["out"]
    return out, res


def kernel(data, targets, task_pool):
    out, _ = run(data, targets, task_pool)
    return out
